# revision 13
# baseline (speedup 1.0000x reference)
"""GCN edge-prediction kernel for 8 Trainium2 NeuronCores.

Strategy (per sharding hint): nodes (and segment_sum outputs) are sharded
contiguously across the 8 cores; each GCN layer all-gathers the
degree-scaled transformed features (the halo exchange, which for a random
graph is everything), then each core gathers its own edges' source rows via
batched SWDGE dma_gather and scatter-adds them with one-hot selector
matmuls on the tensor engine. The edge-pair MLP head is data-parallel over
edge pairs. Small weight matrices are replicated.
"""

import os
import sys
import types

import numpy as np
import ml_dtypes

import concourse.bacc as bacc
import concourse.bass as bass
import concourse.mybir as mybir
import concourse.tile as tile
from concourse.vector_clock import ScopedClock
from concourse.bass_utils import run_bass_kernel_spmd
from concourse.library_config import mlp as _mlp_library

BF16 = ml_dtypes.bfloat16
LAST_EXEC_NS = None

# ----------------------------------------------------------------------------
# Workaround: walrus rejects instructions with more than a few sem waits; the
# TileContext tail drain accumulates one wait per logical processor. Split
# them across preceding sync-engine nops (1 wait each).


def _patched_drain_and_barrier(self, tick_clock, wait_clock):
    nops = [self.nc.sync.nop(nofuse=True) for _ in range(30)]
    drain_inst = self.nc.sync.drain()
    wait_clock.add_sem_waits(
        drain_inst.ins, ScopedClock({None: tick_clock.global_clock})
    )
    si = drain_inst.ins.sync_info
    waits = list(si.on_wait) if si and si.on_wait else []
    if waits:
        chunks = [waits[i : i + 1] for i in range(0, len(waits), 1)]
        assert len(chunks) <= len(nops), f"too many wait chunks: {len(chunks)}"
        for nop_inst, chunk in zip(nops, chunks):
            nsi = nop_inst.ins.sync_info
            if nsi is None:
                nop_inst.ins.sync_info = mybir.SyncInfo(on_wait=chunk, on_update=[])
            else:
                nsi.on_wait = chunk
        si.on_wait = []
    self.nc.all_engine_barrier()
    popped = self.nc._tile_sem_poison_stack.pop()
    assert popped is self._sem_poison
    self.nc.clear_and_free_semaphores(list(self.sems.allocated().values()))
    self.nc.all_engine_barrier()


tile.TileContext._drain_and_barrier = _patched_drain_and_barrier

# ----------------------------------------------------------------------------
# Workaround 2: Tile assigns SWDGE completion-sem lanes round-robin without
# regard to the SWDGE queue an instruction targets, but a lane's semaphore is
# locked to one queue. Pin dma_gather lanes to their queue_num (lanes 0-3) and
# keep other Pool DMAs on lanes 4-7.

from concourse import tile_sem_assignment as _tsa

_orig_assign_tick = _tsa.TileClockTick._assign_tick


def _patched_assign_tick(self, inst):
    if (
        isinstance(inst, _tsa.DMAInst)
        and inst.engine == mybir.EngineType.Pool
        and not isinstance(inst, _tsa.bass_isa.UserSyncedRemoteDMADescs)
    ):
        if isinstance(inst, mybir.InstDMAGatherAnt):
            self.next_sw_dma_idx = (getattr(inst, "queue_num", 0) or 0) % 4
        else:
            alt = getattr(self, "_np_alt", 0)
            self._np_alt = alt + 1
            self.next_sw_dma_idx = 4 + alt % 4
    return _orig_assign_tick(self, inst)


_tsa.TileClockTick._assign_tick = _patched_assign_tick

# ----------------------------------------------------------------------------
# Configuration

P = 128


class Cfg:
    def __init__(self, n_nodes, n_cores, tiles_per_core, f, nidx, hi_base, lo_lim):
        self.N = n_nodes
        self.NC = n_cores
        self.TPC = tiles_per_core
        self.SHARD = tiles_per_core * P
        self.NPAD = self.SHARD * n_cores
        self.F = f
        self.NIDX = nidx
        self.HI_BASE = hi_base  # base row offset of the "hi" gather window
        self.LO_LIM = lo_lim    # node ids < LO_LIM use the lo window
        self.WLO = min(32768, self.NPAD)
        self.WHI = self.NPAD - hi_base
        assert self.NPAD >= n_nodes
        assert lo_lim <= self.WLO
        assert self.WHI <= 32768


def full_cfg():
    return Cfg(n_nodes=50000, n_cores=8, tiles_per_core=49, f=128, nidx=512,
               hi_base=50176 - 32768, lo_lim=32768)


# ----------------------------------------------------------------------------
# Host-side planning: uniform SPMD structure + per-core index/selector data


def _ceil_to(x, m):
    return (x + m - 1) // m * m


class Plan:
    pass


def make_plan(cfg, edge_index, pairs):
    """edge_index: [2, E] int; pairs: [NP, 2] int (edge-MLP endpoint pairs)."""
    pl = Plan()
    NC, TPC, SHARD, NIDX = cfg.NC, cfg.TPC, cfg.SHARD, cfg.NIDX
    src = np.asarray(edge_index[0], dtype=np.int64)
    dst = np.asarray(edge_index[1], dtype=np.int64)

    core_of = dst // SHARD
    per_core = []
    for c in range(NC):
        m = core_of == c
        s, d = src[m], dst[m]
        tloc = (d - c * SHARD) // P
        nloc = (d - c * SHARD) % P
        hi = (s >= cfg.LO_LIM).astype(np.int64)
        order = np.lexsort((s, hi, tloc))
        per_core.append((s[order], tloc[order], nloc[order], hi[order]))

    # uniform per-(tile,class) block counts
    def blkmax(cls):
        mx = 1
        for c in range(NC):
            s, tloc, nloc, hi = per_core[c]
            for t in range(TPC):
                n = int(np.sum((tloc == t) & (hi == cls)))
                mx = max(mx, (n + P - 1) // P)
        return mx

    BLO, BHI = blkmax(0), blkmax(1)
    pl.BLO, pl.BHI = BLO, BHI
    pl.NBLK = TPC * (BLO + BHI)
    stream_lo = TPC * BLO * P
    stream_hi = TPC * BHI * P
    pl.CALLS_LO = (stream_lo + NIDX - 1) // NIDX
    pl.CALLS_HI = (stream_hi + NIDX - 1) // NIDX

    pl.gidx_lo = []
    pl.gidx_hi = []
    pl.onehot = []
    for c in range(NC):
        s, tloc, nloc, hi = per_core[c]
        idx_lo = np.zeros(pl.CALLS_LO * NIDX, dtype=np.int64)
        idx_hi = np.zeros(pl.CALLS_HI * NIDX, dtype=np.int64)
        oh = np.zeros((pl.NBLK * P, P), dtype=np.float32)
        for t in range(TPC):
            for cls in (0, 1):
                m = (tloc == t) & (hi == cls)
                ss, nn = s[m], nloc[m]
                k = np.arange(len(ss))
                if cls == 0:
                    idx_lo[t * BLO * P + k] = ss
                    blk = t * (BLO + BHI) + k // P
                else:
                    idx_hi[t * BHI * P + k] = ss - cfg.HI_BASE
                    blk = t * (BLO + BHI) + BLO + k // P
                oh[blk * P + k % P, nn] = 1.0
        pl.gidx_lo.append(_wrap_idx(idx_lo, NIDX))
        pl.gidx_hi.append(_wrap_idx(idx_hi, NIDX))
        pl.onehot.append(oh.astype(BF16))

    # ---- head: pair classes by (e0 hi, e1 hi), padded to uniform sizes
    NP_ = pairs.shape[0]
    assert NP_ % NC == 0
    PPC = NP_ // NC
    pl.PPC = PPC
    e0 = np.asarray(pairs[:, 0], dtype=np.int64).reshape(NC, PPC)
    e1 = np.asarray(pairs[:, 1], dtype=np.int64).reshape(NC, PPC)
    cls_all = 2 * (e0 >= cfg.LO_LIM) + (e1 >= cfg.LO_LIM)
    counts = np.zeros((NC, 4), dtype=np.int64)
    for c in range(NC):
        for k in range(4):
            counts[c, k] = np.sum(cls_all[c] == k)
    pl.CLS_PAD = [_ceil_to(int(counts[:, k].max()), NIDX) for k in range(4)]
    pl.HTOT = int(sum(pl.CLS_PAD))
    pl.PT = pl.HTOT // P
    pl.HCALLS = pl.HTOT // NIDX
    # per-call (base0, base1) selection: class k -> e0 base = k >> 1, e1 = k & 1
    pl.hcall_cls = []
    for k in range(4):
        pl.hcall_cls += [k] * (pl.CLS_PAD[k] // NIDX)
    pl.hidx0 = []
    pl.hidx1 = []
    pl.hperm = []
    pl.hcounts = counts
    for c in range(NC):
        i0 = np.zeros(pl.HTOT, dtype=np.int64)
        i1 = np.zeros(pl.HTOT, dtype=np.int64)
        order = np.argsort(cls_all[c], kind="stable")
        pl.hperm.append(order)
        off = 0
        pos = 0
        for k in range(4):
            n = int(counts[c, k])
            sel = order[pos : pos + n]
            a0 = e0[c][sel] - (cfg.HI_BASE if k >= 2 else 0)
            a1 = e1[c][sel] - (cfg.HI_BASE if (k & 1) else 0)
            i0[off : off + n] = a0
            i1[off : off + n] = a1
            off += pl.CLS_PAD[k]
            pos += n
        pl.hidx0.append(_wrap_idx(i0, NIDX))
        pl.hidx1.append(_wrap_idx(i1, NIDX))
    return pl


def _wrap_idx(flat, nidx):
    """[L] -> [128, (L/nidx)*(nidx/16)] int16 in dma_gather's wrapped layout."""
    assert len(flat) % nidx == 0
    ncall = len(flat) // nidx
    w = nidx // 16
    out = np.zeros((16, ncall * w), dtype=np.int16)
    for b in range(ncall):
        out[:, b * w : (b + 1) * w] = flat[b * nidx : (b + 1) * nidx].reshape(w, 16).T
    return np.tile(out, (8, 1))


# ----------------------------------------------------------------------------
# Bass program


def build_bass(cfg, pl, trace_friendly=False):
    NC, TPC, F, NIDX = cfg.NC, cfg.TPC, cfg.F, cfg.NIDX
    NPAD = cfg.NPAD
    SHARD = cfg.SHARD
    BLO, BHI = pl.BLO, pl.BHI
    NBT = BLO + BHI
    f32 = mybir.dt.float32
    bf16 = mybir.dt.bfloat16
    i16 = mybir.dt.int16

    nc = bacc.Bacc("TRN2", num_swdge_queues=4)

    # ---- dram inputs
    xT = nc.dram_tensor("xT", [P, SHARD], bf16, kind="ExternalInput")
    Ws = [nc.dram_tensor(f"W{i}", [P, P], bf16, kind="ExternalInput") for i in range(4)]
    bs = [nc.dram_tensor(f"b{i}", [P, 1], f32, kind="ExternalInput") for i in range(4)]
    Wl1 = nc.dram_tensor("Wl1", [2 * P, P], bf16, kind="ExternalInput")
    bl1 = nc.dram_tensor("bl1", [P, 1], f32, kind="ExternalInput")
    Wl2 = nc.dram_tensor("Wl2", [P, 1], bf16, kind="ExternalInput")
    bl2 = nc.dram_tensor("bl2", [1, 1], f32, kind="ExternalInput")
    oh_d = nc.dram_tensor("onehot", [pl.NBLK * P, P], bf16, kind="ExternalInput")
    id_d = nc.dram_tensor("id128", [P, P], bf16, kind="ExternalInput")
    gil_d = nc.dram_tensor("gidx_lo", [P, pl.CALLS_LO * (NIDX // 16)], i16,
                           kind="ExternalInput")
    gih_d = nc.dram_tensor("gidx_hi", [P, pl.CALLS_HI * (NIDX // 16)], i16,
                           kind="ExternalInput")
    hi0_d = nc.dram_tensor("hidx0", [P, pl.HCALLS * (NIDX // 16)], i16,
                           kind="ExternalInput")
    hi1_d = nc.dram_tensor("hidx1", [P, pl.HCALLS * (NIDX // 16)], i16,
                           kind="ExternalInput")
    zout = nc.dram_tensor("zout", [1, pl.HTOT], f32, kind="ExternalOutput")

    oh_v = oh_d.ap().rearrange("(b p) n -> p b n", p=P)  # [128, NBLK, 128]

    with tile.TileContext(nc) as tc:
        nc.gpsimd.load_library(_mlp_library)
        nidx_reg = nc.gpsimd.to_reg(NIDX)
        with (
            tc.tile_pool(name="resident", bufs=1) as rp,
            tc.tile_pool(name="hA", bufs=TPC) as hA,
            tc.tile_pool(name="hB", bufs=TPC) as hB,
            tc.tile_pool(name="hwp", bufs=TPC) as hwp,
            tc.tile_pool(name="oh", bufs=3) as ohp,
            tc.tile_pool(name="g", bufs=6) as gpool,
            tc.tile_pool(name="work", bufs=4) as wk,
            tc.tile_pool(name="zp", bufs=8) as zp,
            tc.tile_pool(name="psum", bufs=3, space="PSUM") as pp,
            tc.tile_pool(name="pst", bufs=2, space="PSUM") as pst,
            tc.tile_pool(name="psrow", bufs=2, space="PSUM") as ppr,
            tc.tile_pool(name="dram", bufs=2, space="DRAM") as dp,
        ):
            # ---------- load resident tensors
            w_t = []
            for i in range(4):
                w = rp.tile([P, P], bf16, tag=f"W{i}")
                nc.sync.dma_start(out=w[:], in_=Ws[i][:, :])
                w_t.append(w)
            b_t = []
            for i in range(4):
                b = rp.tile([P, 1], f32, tag=f"b{i}")
                nc.sync.dma_start(out=b[:], in_=bs[i][:, :])
                b_t.append(b)
            wl1_t = rp.tile([P, 2 * P], bf16, tag="Wl1")
            nc.sync.dma_start(
                out=wl1_t[:].rearrange("p (k q) -> p k q", k=2),
                in_=Wl1.ap().rearrange("(k p) q -> p k q", p=P),
            )
            bl1_t = rp.tile([P, 1], f32, tag="bl1")
            nc.sync.dma_start(out=bl1_t[:], in_=bl1[:, :])
            wl2_t = rp.tile([P, 1], bf16, tag="Wl2")
            nc.sync.dma_start(out=wl2_t[:], in_=Wl2[:, :])
            bl2_t = rp.tile([1, 1], f32, tag="bl2")
            nc.sync.dma_start(out=bl2_t[:], in_=bl2[:, :])
            gil_t = rp.tile([P, pl.CALLS_LO * (NIDX // 16)], i16, tag="gil")
            nc.sync.dma_start(out=gil_t[:], in_=gil_d[:, :])
            gih_t = rp.tile([P, pl.CALLS_HI * (NIDX // 16)], i16, tag="gih")
            nc.sync.dma_start(out=gih_t[:], in_=gih_d[:, :])
            hi0_t = rp.tile([P, pl.HCALLS * (NIDX // 16)], i16, tag="hi0")
            nc.sync.dma_start(out=hi0_t[:], in_=hi0_d[:, :])
            hi1_t = rp.tile([P, pl.HCALLS * (NIDX // 16)], i16, tag="hi1")
            nc.sync.dma_start(out=hi1_t[:], in_=hi1_d[:, :])
            ones_e = rp.tile([P, 1], bf16, tag="ones_e")
            nc.gpsimd.memset(ones_e[:], 1.0)
            ones_k1 = rp.tile([1, P], f32, tag="ones_k1")
            nc.gpsimd.memset(ones_k1[:], 1.0)
            id_t = rp.tile([P, P], bf16, tag="id128")
            nc.sync.dma_start(out=id_t[:], in_=id_d[:, :])

            # initial h (own shard, feature-major)
            h_cur = []
            for t in range(TPC):
                ht = hA.tile([P, P], bf16, tag="h")
                nc.sync.dma_start(out=ht[:], in_=xT[:, t * P : (t + 1) * P])
                h_cur.append(ht)

            # ---------- degree pass: deg[node] = sum_e onehot[e, node]
            deg_sb = rp.tile([1, SHARD], f32, tag="deg")
            for t in range(TPC):
                oh_t = ohp.tile([P, NBT * P], bf16, tag="oh")
                nc.sync.dma_start(
                    out=oh_t[:].rearrange("p (b n) -> p b n", n=P),
                    in_=oh_v[:, t * NBT : (t + 1) * NBT, :],
                )
                dps = ppr.tile([1, P], f32, tag="row")
                for j in range(NBT):
                    nc.tensor.matmul(
                        out=dps[:],
                        lhsT=ones_e[:],
                        rhs=oh_t[:, j * P : (j + 1) * P],
                        start=(j == 0),
                        stop=(j == NBT - 1),
                    )
                nc.vector.tensor_copy(out=deg_sb[0:1, t * P : (t + 1) * P], in_=dps[:])
            # dis = sqrt(1/(deg+1)) broadcast to all partitions (in-place chain)
            nc.vector.tensor_scalar_add(deg_sb[:], deg_sb[:], 1.0)
            nc.vector.reciprocal(deg_sb[:], deg_sb[:])
            nc.scalar.activation(deg_sb[:], deg_sb[:], mybir.ActivationFunctionType.Sqrt)
            dis_bc = rp.tile([P, SHARD], f32, tag="dis_bc")
            for o in range(0, SHARD, 512):
                w = min(512, SHARD - o)
                bps = pp.tile([P, 512], f32, tag="mm")
                nc.tensor.matmul(out=bps[:, :w], lhsT=ones_k1[:],
                                 rhs=deg_sb[0:1, o : o + w], start=True, stop=True)
                nc.vector.tensor_copy(out=dis_bc[:, o : o + w], in_=bps[:, :w])

            # ---------- GCN layers
            hpools = [hA, hB]
            for layer in range(4):
                # phase A: hw' = (h @ W) * dis, transpose to node-major, stage AG in
                ag_in = dp.tile([SHARD, P], bf16, tag="ag_in")
                ag_out = dp.tile([NPAD, P], bf16, tag="ag_out")
                hw_tiles = []
                for t in range(TPC):
                    mm = pp.tile([P, P], f32, tag="mm")
                    nc.tensor.matmul(out=mm[:], lhsT=w_t[layer][:], rhs=h_cur[t][:],
                                     start=True, stop=True)
                    hw = hwp.tile([P, P], bf16, tag="hw")
                    nc.vector.tensor_tensor(
                        out=hw[:], in0=mm[:], in1=dis_bc[:, t * P : (t + 1) * P],
                        op=mybir.AluOpType.mult,
                    )
                    hw_tiles.append(hw)
                    tp = pst.tile([P, P], bf16, tag="mmt")
                    nc.tensor.transpose(out=tp[:], in_=hw[:], identity=id_t[:])
                    hwn = wk.tile([P, P], bf16, tag="hwn")
                    nc.vector.tensor_copy(out=hwn[:], in_=tp[:])
                    nc.sync.dma_start(out=ag_in[t * P : (t + 1) * P, :], in_=hwn[:])
                nc.gpsimd.collective_compute(
                    "AllGather",
                    mybir.AluOpType.bypass,
                    replica_groups=[list(range(NC))],
                    ins=[ag_in[:].opt()],
                    outs=[ag_out[:].opt()],
                )

                # phase C: gather + scatter-matmul + epilogue
                gat = {}

                def get_gather(stream, call):
                    key = (layer, stream, call)
                    if key in gat:
                        return gat[key]
                    g = gpool.tile([P, NIDX // P, F], bf16, tag="g")
                    if stream == 0:
                        base, win, it = 0, cfg.WLO, gil_t
                    else:
                        base, win, it = cfg.HI_BASE, cfg.WHI, gih_t
                    w = NIDX // 16
                    nc.gpsimd.dma_gather(
                        g[:],
                        ag_out[base : base + win, :],
                        it[:, call * w : (call + 1) * w],
                        NIDX, nidx_reg, F,
                        queue_num=(call % 4),
                    )
                    gat[key] = g
                    return g

                for t in range(TPC):
                    oh_t = ohp.tile([P, NBT * P], bf16, tag="oh")
                    nc.sync.dma_start(
                        out=oh_t[:].rearrange("p (b n) -> p b n", n=P),
                        in_=oh_v[:, t * NBT : (t + 1) * NBT, :],
                    )
                    agg = pp.tile([P, P], f32, tag="mm")
                    nb = 0
                    for cls in (0, 1):
                        BU = BLO if cls == 0 else BHI
                        for j in range(BU):
                            pos = (t * BU + j) * P
                            g = get_gather(cls, pos // NIDX)
                            sl = (pos % NIDX) // P
                            nc.tensor.matmul(
                                out=agg[:],
                                lhsT=g[:, sl, :],
                                rhs=oh_t[:, (cls * BLO + j) * P : (cls * BLO + j + 1) * P],
                                start=(nb == 0),
                                stop=(nb == NBT - 1),
                            )
                            nb += 1
                    # epilogue: h' = act(dis * (agg + hw') + b)
                    s1 = wk.tile([P, P], f32, tag="s1")
                    nc.vector.tensor_tensor(out=s1[:], in0=agg[:], in1=hw_tiles[t][:],
                                            op=mybir.AluOpType.add)
                    s2 = wk.tile([P, P], f32, tag="s2")
                    nc.vector.tensor_tensor(out=s2[:], in0=s1[:],
                                            in1=dis_bc[:, t * P : (t + 1) * P],
                                            op=mybir.AluOpType.mult)
                    hn = hpools[(layer + 1) % 2].tile([P, P], bf16, tag="h")
                    func = (mybir.ActivationFunctionType.Relu if layer < 3
                            else mybir.ActivationFunctionType.Identity)
                    nc.scalar.activation(hn[:], s2[:], func, bias=b_t[layer][:])
                    h_cur[t] = hn

            # ---------- final AG of h4 (node-major) for the head
            h4_in = dp.tile([SHARD, P], bf16, tag="ag_in")
            h4_tab = dp.tile([NPAD, P], bf16, tag="ag_out")
            for t in range(TPC):
                tp4 = pst.tile([P, P], bf16, tag="mmt")
                nc.tensor.transpose(out=tp4[:], in_=h_cur[t][:], identity=id_t[:])
                hn4 = wk.tile([P, P], bf16, tag="hwn")
                nc.vector.tensor_copy(out=hn4[:], in_=tp4[:])
                nc.sync.dma_start(out=h4_in[t * P : (t + 1) * P, :], in_=hn4[:])
            nc.gpsimd.collective_compute(
                "AllGather",
                mybir.AluOpType.bypass,
                replica_groups=[list(range(NC))],
                ins=[h4_in[:].opt()],
                outs=[h4_tab[:].opt()],
            )

            # ---------- head MLP over pair tiles
            hgat = {}

            def get_hgather(which, call):
                key = (which, call)
                if key in hgat:
                    return hgat[key]
                g = gpool.tile([P, 1, NIDX], bf16, tag="hg")
                cls = pl.hcall_cls[call]
                hi_sel = (cls >> 1) if which == 0 else (cls & 1)
                base = cfg.HI_BASE if hi_sel else 0
                win = cfg.WHI if hi_sel else cfg.WLO
                it = hi0_t if which == 0 else hi1_t
                w = NIDX // 16
                nc.gpsimd.dma_gather(
                    g[:],
                    h4_tab[base : base + win, :],
                    it[:, call * w : (call + 1) * w],
                    NIDX, nidx_reg, F,
                    transpose=True,
                    queue_num=(call % 4),
                )
                hgat[key] = g
                return g

            for pt in range(pl.PT):
                call = pt * P // NIDX
                sl = (pt * P % NIDX)
                g0 = get_hgather(0, call)
                g1 = get_hgather(1, call)
                z1p = pp.tile([P, P], f32, tag="mm")
                nc.tensor.matmul(out=z1p[:], lhsT=wl1_t[:, 0:P],
                                 rhs=g0[:, 0, sl : sl + P], start=True, stop=False)
                nc.tensor.matmul(out=z1p[:], lhsT=wl1_t[:, P : 2 * P],
                                 rhs=g1[:, 0, sl : sl + P], start=False, stop=True)
                z1 = wk.tile([P, P], bf16, tag="z1")
                nc.scalar.activation(z1[:], z1p[:], mybir.ActivationFunctionType.Relu,
                                     bias=bl1_t[:])
                z2p = ppr.tile([1, P], f32, tag="row")
                nc.tensor.matmul(out=z2p[:], lhsT=wl2_t[:], rhs=z1[:],
                                 start=True, stop=True)
                zrow = zp.tile([1, P], f32, tag="z")
                nc.vector.tensor_tensor(out=zrow[:], in0=z2p[:],
                                        in1=bl2_t[:].to_broadcast([1, P]),
                                        op=mybir.AluOpType.add)
                nc.sync.dma_start(out=zout[0:1, pt * P : (pt + 1) * P], in_=zrow[:])
    nc.compile()
    return nc


# ----------------------------------------------------------------------------
# Host wrapper


def _prep_inputs(cfg, pl, x, weights, core):
    (W0, b0, W1, b1, W2, b2, W3, b3, Wl1, bl1, Wl2, bl2) = weights
    SHARD = cfg.SHARD
    xp = np.zeros((cfg.NPAD, cfg.F), dtype=np.float32)
    xp[: x.shape[0]] = x
    xT = xp[core * SHARD : (core + 1) * SHARD].T.astype(BF16)
    m = {
        "xT": np.ascontiguousarray(xT),
        "W0": W0.astype(BF16), "W1": W1.astype(BF16),
        "W2": W2.astype(BF16), "W3": W3.astype(BF16),
        "b0": b0.reshape(-1, 1).astype(np.float32),
        "b1": b1.reshape(-1, 1).astype(np.float32),
        "b2": b2.reshape(-1, 1).astype(np.float32),
        "b3": b3.reshape(-1, 1).astype(np.float32),
        "Wl1": Wl1.astype(BF16),
        "bl1": bl1.reshape(-1, 1).astype(np.float32),
        "Wl2": Wl2.reshape(-1, 1).astype(BF16),
        "bl2": bl2.reshape(1, 1).astype(np.float32),
        "onehot": pl.onehot[core],
        "gidx_lo": pl.gidx_lo[core],
        "gidx_hi": pl.gidx_hi[core],
        "id128": np.eye(cfg.F, dtype=np.float32).astype(BF16),
        "hidx0": pl.hidx0[core],
        "hidx1": pl.hidx1[core],
    }
    return m


def _unpack_head(cfg, pl, zouts):
    """Per-core zout [1, HTOT] -> global z [NP] in original pair order."""
    zs = []
    for c in range(cfg.NC):
        z = zouts[c].reshape(-1)
        parts = []
        off = 0
        for k in range(4):
            n = int(pl.hcounts[c, k])
            parts.append(z[off : off + n])
            off += pl.CLS_PAD[k]
        zc = np.concatenate(parts)
        orig = np.empty(pl.PPC, dtype=np.float32)
        orig[pl.hperm[c]] = zc
        zs.append(orig)
    return np.concatenate(zs)


def run(cfg, x, edge_index, pairs, weights, trace=False):
    pl = make_plan(cfg, edge_index, pairs)
    nc = build_bass(cfg, pl)
    in_maps = [_prep_inputs(cfg, pl, x, weights, c) for c in range(cfg.NC)]
    res = run_bass_kernel_spmd(nc, in_maps, core_ids=list(range(cfg.NC)), trace=trace)
    z = _unpack_head(cfg, pl, [res.results[c]["zout"] for c in range(cfg.NC)])
    return z, res


def kernel(x, edge_index, pos_edges_train, neg_edges_train, pos_edges_test,
           neg_edges_test, W0, b0, W1, b1, W2, b2, W3, b3, Wl1, bl1, Wl2, bl2):
    cfg = full_cfg()
    pairs = np.concatenate([
        np.asarray(pos_edges_train).T, np.asarray(neg_edges_train).T,
        np.asarray(pos_edges_test).T, np.asarray(neg_edges_test).T,
    ], axis=0)
    weights = (np.asarray(W0), np.asarray(b0), np.asarray(W1), np.asarray(b1),
               np.asarray(W2), np.asarray(b2), np.asarray(W3), np.asarray(b3),
               np.asarray(Wl1), np.asarray(bl1), np.asarray(Wl2), np.asarray(bl2))
    trace = bool(int(os.environ.get("GCN_TRACE", "0")))
    z, res = run(cfg, np.asarray(x), np.asarray(edge_index), pairs, weights,
                 trace=trace)
    global LAST_EXEC_NS
    LAST_EXEC_NS = res.exec_time_ns
    n_train = pos_edges_train.shape[1] + neg_edges_train.shape[1]
    return z[:n_train].astype(np.float32), z[n_train:].astype(np.float32)


# revision 15
# speedup vs baseline: 1.0683x; 1.0683x over previous
"""GCN edge-prediction kernel for 8 Trainium2 NeuronCores.

Strategy (per sharding hint): nodes (and segment_sum outputs) are sharded
contiguously across the 8 cores; each GCN layer all-gathers the
degree-scaled transformed features (the halo exchange, which for a random
graph is everything), then each core gathers its own edges' source rows via
batched SWDGE dma_gather and scatter-adds them with one-hot selector
matmuls on the tensor engine. The edge-pair MLP head is data-parallel over
edge pairs. Small weight matrices are replicated.
"""

import os
import sys
import types

import numpy as np
import ml_dtypes

import concourse.bacc as bacc
import concourse.bass as bass
import concourse.mybir as mybir
import concourse.tile as tile
from concourse.vector_clock import ScopedClock
from concourse.bass_utils import run_bass_kernel_spmd
from concourse.library_config import mlp as _mlp_library

BF16 = ml_dtypes.bfloat16
LAST_EXEC_NS = None

# ----------------------------------------------------------------------------
# Workaround: walrus rejects instructions with more than a few sem waits; the
# TileContext tail drain accumulates one wait per logical processor. Split
# them across preceding sync-engine nops (1 wait each).


def _patched_drain_and_barrier(self, tick_clock, wait_clock):
    nops = [self.nc.sync.nop(nofuse=True) for _ in range(30)]
    drain_inst = self.nc.sync.drain()
    wait_clock.add_sem_waits(
        drain_inst.ins, ScopedClock({None: tick_clock.global_clock})
    )
    si = drain_inst.ins.sync_info
    waits = list(si.on_wait) if si and si.on_wait else []
    if waits:
        chunks = [waits[i : i + 1] for i in range(0, len(waits), 1)]
        assert len(chunks) <= len(nops), f"too many wait chunks: {len(chunks)}"
        for nop_inst, chunk in zip(nops, chunks):
            nsi = nop_inst.ins.sync_info
            if nsi is None:
                nop_inst.ins.sync_info = mybir.SyncInfo(on_wait=chunk, on_update=[])
            else:
                nsi.on_wait = chunk
        si.on_wait = []
    self.nc.all_engine_barrier()
    popped = self.nc._tile_sem_poison_stack.pop()
    assert popped is self._sem_poison
    self.nc.clear_and_free_semaphores(list(self.sems.allocated().values()))
    self.nc.all_engine_barrier()


tile.TileContext._drain_and_barrier = _patched_drain_and_barrier

# ----------------------------------------------------------------------------
# Workaround 2: Tile assigns SWDGE completion-sem lanes round-robin without
# regard to the SWDGE queue an instruction targets, but a lane's semaphore is
# locked to one queue. Pin dma_gather lanes to their queue_num (lanes 0-3) and
# keep other Pool DMAs on lanes 4-7.

from concourse import tile_sem_assignment as _tsa

_orig_assign_tick = _tsa.TileClockTick._assign_tick


def _patched_assign_tick(self, inst):
    if (
        isinstance(inst, _tsa.DMAInst)
        and inst.engine == mybir.EngineType.Pool
        and not isinstance(inst, _tsa.bass_isa.UserSyncedRemoteDMADescs)
    ):
        if isinstance(inst, mybir.InstDMAGatherAnt):
            self.next_sw_dma_idx = (getattr(inst, "queue_num", 0) or 0) % 4
        else:
            alt = getattr(self, "_np_alt", 0)
            self._np_alt = alt + 1
            self.next_sw_dma_idx = 4 + alt % 4
    return _orig_assign_tick(self, inst)


_tsa.TileClockTick._assign_tick = _patched_assign_tick

# ----------------------------------------------------------------------------
# Configuration

P = 128


class Cfg:
    def __init__(self, n_nodes, n_cores, tiles_per_core, f, nidx, hi_base, lo_lim,
                 nidxh=None):
        self.N = n_nodes
        self.NC = n_cores
        self.TPC = tiles_per_core
        self.SHARD = tiles_per_core * P
        self.NPAD = self.SHARD * n_cores
        self.F = f
        self.NIDX = nidx
        self.NIDXH = nidxh or nidx
        self.HI_BASE = hi_base  # base row offset of the "hi" gather window
        self.LO_LIM = lo_lim    # node ids < LO_LIM use the lo window
        self.WLO = min(32768, self.NPAD)
        self.WHI = self.NPAD - hi_base
        assert self.NPAD >= n_nodes
        assert lo_lim <= self.WLO
        assert self.WHI <= 32768


def full_cfg():
    return Cfg(n_nodes=50000, n_cores=8, tiles_per_core=49, f=128, nidx=1024,
               hi_base=50176 - 32768, lo_lim=32768, nidxh=512)


# ----------------------------------------------------------------------------
# Host-side planning: uniform SPMD structure + per-core index/selector data


def _ceil_to(x, m):
    return (x + m - 1) // m * m


class Plan:
    pass


def make_plan(cfg, edge_index, pairs):
    """edge_index: [2, E] int; pairs: [NP, 2] int (edge-MLP endpoint pairs)."""
    pl = Plan()
    NC, TPC, SHARD, NIDX = cfg.NC, cfg.TPC, cfg.SHARD, cfg.NIDX
    src = np.asarray(edge_index[0], dtype=np.int64)
    dst = np.asarray(edge_index[1], dtype=np.int64)

    core_of = dst // SHARD
    per_core = []
    for c in range(NC):
        m = core_of == c
        s, d = src[m], dst[m]
        tloc = (d - c * SHARD) // P
        nloc = (d - c * SHARD) % P
        hi = (s >= cfg.LO_LIM).astype(np.int64)
        order = np.lexsort((s, hi, tloc))
        per_core.append((s[order], tloc[order], nloc[order], hi[order]))

    # uniform per-(tile,class) block counts
    def blkmax(cls):
        mx = 1
        for c in range(NC):
            s, tloc, nloc, hi = per_core[c]
            for t in range(TPC):
                n = int(np.sum((tloc == t) & (hi == cls)))
                mx = max(mx, (n + P - 1) // P)
        return mx

    BLO, BHI = blkmax(0), blkmax(1)
    pl.BLO, pl.BHI = BLO, BHI
    pl.NBLK = TPC * (BLO + BHI)
    stream_lo = TPC * BLO * P
    stream_hi = TPC * BHI * P
    pl.CALLS_LO = (stream_lo + NIDX - 1) // NIDX
    pl.CALLS_HI = (stream_hi + NIDX - 1) // NIDX

    pl.gidx_lo = []
    pl.gidx_hi = []
    pl.onehot = []
    for c in range(NC):
        s, tloc, nloc, hi = per_core[c]
        idx_lo = np.zeros(pl.CALLS_LO * NIDX, dtype=np.int64)
        idx_hi = np.zeros(pl.CALLS_HI * NIDX, dtype=np.int64)
        oh = np.zeros((pl.NBLK * P, P), dtype=np.float32)
        for t in range(TPC):
            for cls in (0, 1):
                m = (tloc == t) & (hi == cls)
                ss, nn = s[m], nloc[m]
                k = np.arange(len(ss))
                if cls == 0:
                    idx_lo[t * BLO * P + k] = ss
                    blk = t * (BLO + BHI) + k // P
                else:
                    idx_hi[t * BHI * P + k] = ss - cfg.HI_BASE
                    blk = t * (BLO + BHI) + BLO + k // P
                oh[blk * P + k % P, nn] = 1.0
        pl.gidx_lo.append(_wrap_idx(idx_lo, NIDX))
        pl.gidx_hi.append(_wrap_idx(idx_hi, NIDX))
        pl.onehot.append(oh.astype(BF16))

    # ---- head: pair classes by (e0 hi, e1 hi), padded to uniform sizes
    NP_ = pairs.shape[0]
    assert NP_ % NC == 0
    PPC = NP_ // NC
    pl.PPC = PPC
    e0 = np.asarray(pairs[:, 0], dtype=np.int64).reshape(NC, PPC)
    e1 = np.asarray(pairs[:, 1], dtype=np.int64).reshape(NC, PPC)
    cls_all = 2 * (e0 >= cfg.LO_LIM) + (e1 >= cfg.LO_LIM)
    counts = np.zeros((NC, 4), dtype=np.int64)
    for c in range(NC):
        for k in range(4):
            counts[c, k] = np.sum(cls_all[c] == k)
    NIDXH = cfg.NIDXH
    pl.CLS_PAD = [_ceil_to(int(counts[:, k].max()), NIDXH) for k in range(4)]
    pl.HTOT = int(sum(pl.CLS_PAD))
    pl.PT = pl.HTOT // P
    pl.HCALLS = pl.HTOT // NIDXH
    # per-call (base0, base1) selection: class k -> e0 base = k >> 1, e1 = k & 1
    pl.hcall_cls = []
    for k in range(4):
        pl.hcall_cls += [k] * (pl.CLS_PAD[k] // NIDXH)
    pl.hidx0 = []
    pl.hidx1 = []
    pl.hperm = []
    pl.hcounts = counts
    for c in range(NC):
        i0 = np.zeros(pl.HTOT, dtype=np.int64)
        i1 = np.zeros(pl.HTOT, dtype=np.int64)
        order = np.argsort(cls_all[c], kind="stable")
        pl.hperm.append(order)
        off = 0
        pos = 0
        for k in range(4):
            n = int(counts[c, k])
            sel = order[pos : pos + n]
            a0 = e0[c][sel] - (cfg.HI_BASE if k >= 2 else 0)
            a1 = e1[c][sel] - (cfg.HI_BASE if (k & 1) else 0)
            i0[off : off + n] = a0
            i1[off : off + n] = a1
            off += pl.CLS_PAD[k]
            pos += n
        pl.hidx0.append(_wrap_idx(i0, NIDXH))
        pl.hidx1.append(_wrap_idx(i1, NIDXH))
    return pl


def _wrap_idx(flat, nidx):
    """[L] -> [128, (L/nidx)*(nidx/16)] int16 in dma_gather's wrapped layout."""
    assert len(flat) % nidx == 0
    ncall = len(flat) // nidx
    w = nidx // 16
    out = np.zeros((16, ncall * w), dtype=np.int16)
    for b in range(ncall):
        out[:, b * w : (b + 1) * w] = flat[b * nidx : (b + 1) * nidx].reshape(w, 16).T
    return np.tile(out, (8, 1))


# ----------------------------------------------------------------------------
# Bass program


def build_bass(cfg, pl, trace_friendly=False):
    NC, TPC, F, NIDX = cfg.NC, cfg.TPC, cfg.F, cfg.NIDX
    NPAD = cfg.NPAD
    SHARD = cfg.SHARD
    BLO, BHI = pl.BLO, pl.BHI
    NBT = BLO + BHI
    f32 = mybir.dt.float32
    bf16 = mybir.dt.bfloat16
    i16 = mybir.dt.int16

    nc = bacc.Bacc("TRN2", num_swdge_queues=4, dynamic_dma_scratch_size=32768)

    # ---- dram inputs
    xT = nc.dram_tensor("xT", [P, SHARD], bf16, kind="ExternalInput")
    Ws = [nc.dram_tensor(f"W{i}", [P, P], bf16, kind="ExternalInput") for i in range(4)]
    bs = [nc.dram_tensor(f"b{i}", [P, 1], f32, kind="ExternalInput") for i in range(4)]
    Wl1 = nc.dram_tensor("Wl1", [2 * P, P], bf16, kind="ExternalInput")
    bl1 = nc.dram_tensor("bl1", [P, 1], f32, kind="ExternalInput")
    Wl2 = nc.dram_tensor("Wl2", [P, 1], bf16, kind="ExternalInput")
    bl2 = nc.dram_tensor("bl2", [1, 1], f32, kind="ExternalInput")
    oh_d = nc.dram_tensor("onehot", [pl.NBLK * P, P], bf16, kind="ExternalInput")
    id_d = nc.dram_tensor("id128", [P, P], bf16, kind="ExternalInput")
    gil_d = nc.dram_tensor("gidx_lo", [P, pl.CALLS_LO * (NIDX // 16)], i16,
                           kind="ExternalInput")
    gih_d = nc.dram_tensor("gidx_hi", [P, pl.CALLS_HI * (NIDX // 16)], i16,
                           kind="ExternalInput")
    NIDXH = cfg.NIDXH
    hi0_d = nc.dram_tensor("hidx0", [P, pl.HCALLS * (NIDXH // 16)], i16,
                           kind="ExternalInput")
    hi1_d = nc.dram_tensor("hidx1", [P, pl.HCALLS * (NIDXH // 16)], i16,
                           kind="ExternalInput")
    zout = nc.dram_tensor("zout", [1, pl.HTOT], f32, kind="ExternalOutput")

    oh_v = oh_d.ap().rearrange("(b p) n -> p b n", p=P)  # [128, NBLK, 128]

    with tile.TileContext(nc) as tc:
        nc.gpsimd.load_library(_mlp_library)
        nidx_reg = nc.gpsimd.to_reg(NIDX)
        nidxh_reg = nc.gpsimd.to_reg(cfg.NIDXH)
        with (
            tc.tile_pool(name="resident", bufs=1) as rp,
            tc.tile_pool(name="hA", bufs=TPC) as hA,
            tc.tile_pool(name="hB", bufs=TPC) as hB,
            tc.tile_pool(name="hwp", bufs=TPC) as hwp,
            tc.tile_pool(name="oh", bufs=3) as ohp,
            tc.tile_pool(name="g", bufs=6) as gpool,
            tc.tile_pool(name="work", bufs=4) as wk,
            tc.tile_pool(name="zp", bufs=8) as zp,
            tc.tile_pool(name="psum", bufs=3, space="PSUM") as pp,
            tc.tile_pool(name="pst", bufs=2, space="PSUM") as pst,
            tc.tile_pool(name="psrow", bufs=2, space="PSUM") as ppr,
            tc.tile_pool(name="dram", bufs=2, space="DRAM") as dp,
        ):
            # ---------- load resident tensors
            w_t = []
            for i in range(4):
                w = rp.tile([P, P], bf16, tag=f"W{i}")
                nc.sync.dma_start(out=w[:], in_=Ws[i][:, :])
                w_t.append(w)
            b_t = []
            for i in range(4):
                b = rp.tile([P, 1], f32, tag=f"b{i}")
                nc.sync.dma_start(out=b[:], in_=bs[i][:, :])
                b_t.append(b)
            wl1_t = rp.tile([P, 2 * P], bf16, tag="Wl1")
            nc.sync.dma_start(
                out=wl1_t[:].rearrange("p (k q) -> p k q", k=2),
                in_=Wl1.ap().rearrange("(k p) q -> p k q", p=P),
            )
            bl1_t = rp.tile([P, 1], f32, tag="bl1")
            nc.sync.dma_start(out=bl1_t[:], in_=bl1[:, :])
            wl2_t = rp.tile([P, 1], bf16, tag="Wl2")
            nc.sync.dma_start(out=wl2_t[:], in_=Wl2[:, :])
            bl2_t = rp.tile([1, 1], f32, tag="bl2")
            nc.sync.dma_start(out=bl2_t[:], in_=bl2[:, :])
            gil_t = rp.tile([P, pl.CALLS_LO * (NIDX // 16)], i16, tag="gil")
            nc.sync.dma_start(out=gil_t[:], in_=gil_d[:, :])
            gih_t = rp.tile([P, pl.CALLS_HI * (NIDX // 16)], i16, tag="gih")
            nc.sync.dma_start(out=gih_t[:], in_=gih_d[:, :])
            hi0_t = rp.tile([P, pl.HCALLS * (NIDXH // 16)], i16, tag="hi0")
            nc.sync.dma_start(out=hi0_t[:], in_=hi0_d[:, :])
            hi1_t = rp.tile([P, pl.HCALLS * (NIDXH // 16)], i16, tag="hi1")
            nc.sync.dma_start(out=hi1_t[:], in_=hi1_d[:, :])
            ones_e = rp.tile([P, 1], bf16, tag="ones_e")
            nc.gpsimd.memset(ones_e[:], 1.0)
            ones_k1 = rp.tile([1, P], f32, tag="ones_k1")
            nc.gpsimd.memset(ones_k1[:], 1.0)
            id_t = rp.tile([P, P], bf16, tag="id128")
            nc.sync.dma_start(out=id_t[:], in_=id_d[:, :])

            # initial h (own shard, feature-major)
            h_cur = []
            for t in range(TPC):
                ht = hA.tile([P, P], bf16, tag="h")
                nc.sync.dma_start(out=ht[:], in_=xT[:, t * P : (t + 1) * P])
                h_cur.append(ht)

            # ---------- degree pass: deg[node] = sum_e onehot[e, node]
            deg_sb = rp.tile([1, SHARD], f32, tag="deg")
            for t in range(TPC):
                oh_t = ohp.tile([P, NBT * P], bf16, tag="oh")
                nc.sync.dma_start(
                    out=oh_t[:].rearrange("p (b n) -> p b n", n=P),
                    in_=oh_v[:, t * NBT : (t + 1) * NBT, :],
                )
                dps = ppr.tile([1, P], f32, tag="row")
                for j in range(NBT):
                    nc.tensor.matmul(
                        out=dps[:],
                        lhsT=ones_e[:],
                        rhs=oh_t[:, j * P : (j + 1) * P],
                        start=(j == 0),
                        stop=(j == NBT - 1),
                    )
                nc.vector.tensor_copy(out=deg_sb[0:1, t * P : (t + 1) * P], in_=dps[:])
            # dis = sqrt(1/(deg+1)) broadcast to all partitions (in-place chain)
            nc.vector.tensor_scalar_add(deg_sb[:], deg_sb[:], 1.0)
            nc.vector.reciprocal(deg_sb[:], deg_sb[:])
            nc.scalar.activation(deg_sb[:], deg_sb[:], mybir.ActivationFunctionType.Sqrt)
            dis_bc = rp.tile([P, SHARD], f32, tag="dis_bc")
            for o in range(0, SHARD, 512):
                w = min(512, SHARD - o)
                bps = pp.tile([P, 512], f32, tag="mm")
                nc.tensor.matmul(out=bps[:, :w], lhsT=ones_k1[:],
                                 rhs=deg_sb[0:1, o : o + w], start=True, stop=True)
                nc.vector.tensor_copy(out=dis_bc[:, o : o + w], in_=bps[:, :w])

            # ---------- GCN layers
            hpools = [hA, hB]
            for layer in range(4):
                # phase A: hw' = (h @ W) * dis, transpose to node-major, stage AG in
                ag_in = dp.tile([SHARD, P], bf16, tag="ag_in")
                ag_out = dp.tile([NPAD, P], bf16, tag="ag_out")
                hw_tiles = []
                for t in range(TPC):
                    mm = pp.tile([P, P], f32, tag="mm")
                    nc.tensor.matmul(out=mm[:], lhsT=w_t[layer][:], rhs=h_cur[t][:],
                                     start=True, stop=True)
                    hw = hwp.tile([P, P], bf16, tag="hw")
                    nc.vector.tensor_tensor(
                        out=hw[:], in0=mm[:], in1=dis_bc[:, t * P : (t + 1) * P],
                        op=mybir.AluOpType.mult,
                    )
                    hw_tiles.append(hw)
                    tp = pst.tile([P, P], bf16, tag="mmt")
                    nc.tensor.transpose(out=tp[:], in_=hw[:], identity=id_t[:])
                    hwn = wk.tile([P, P], bf16, tag="hwn")
                    nc.vector.tensor_copy(out=hwn[:], in_=tp[:])
                    nc.sync.dma_start(out=ag_in[t * P : (t + 1) * P, :], in_=hwn[:])
                nc.gpsimd.collective_compute(
                    "AllGather",
                    mybir.AluOpType.bypass,
                    replica_groups=[list(range(NC))],
                    ins=[ag_in[:].opt()],
                    outs=[ag_out[:].opt()],
                )

                # phase C: gather + scatter-matmul + epilogue
                gat = {}

                def get_gather(stream, call):
                    key = (layer, stream, call)
                    if key in gat:
                        return gat[key]
                    g = gpool.tile([P, NIDX // P, F], bf16, tag="g")
                    if stream == 0:
                        base, win, it = 0, cfg.WLO, gil_t
                    else:
                        base, win, it = cfg.HI_BASE, cfg.WHI, gih_t
                    w = NIDX // 16
                    nc.gpsimd.dma_gather(
                        g[:],
                        ag_out[base : base + win, :],
                        it[:, call * w : (call + 1) * w],
                        NIDX, nidx_reg, F,
                        queue_num=(call % 4),
                    )
                    gat[key] = g
                    return g

                for t in range(TPC):
                    oh_t = ohp.tile([P, NBT * P], bf16, tag="oh")
                    nc.sync.dma_start(
                        out=oh_t[:].rearrange("p (b n) -> p b n", n=P),
                        in_=oh_v[:, t * NBT : (t + 1) * NBT, :],
                    )
                    agg = pp.tile([P, P], f32, tag="mm")
                    nb = 0
                    for cls in (0, 1):
                        BU = BLO if cls == 0 else BHI
                        for j in range(BU):
                            pos = (t * BU + j) * P
                            g = get_gather(cls, pos // NIDX)
                            sl = (pos % NIDX) // P
                            nc.tensor.matmul(
                                out=agg[:],
                                lhsT=g[:, sl, :],
                                rhs=oh_t[:, (cls * BLO + j) * P : (cls * BLO + j + 1) * P],
                                start=(nb == 0),
                                stop=(nb == NBT - 1),
                            )
                            nb += 1
                    # epilogue: h' = act(dis * (agg + hw') + b)
                    s1 = wk.tile([P, P], f32, tag="s1")
                    nc.vector.tensor_tensor(out=s1[:], in0=agg[:], in1=hw_tiles[t][:],
                                            op=mybir.AluOpType.add)
                    s2 = wk.tile([P, P], f32, tag="s2")
                    nc.vector.tensor_tensor(out=s2[:], in0=s1[:],
                                            in1=dis_bc[:, t * P : (t + 1) * P],
                                            op=mybir.AluOpType.mult)
                    hn = hpools[(layer + 1) % 2].tile([P, P], bf16, tag="h")
                    func = (mybir.ActivationFunctionType.Relu if layer < 3
                            else mybir.ActivationFunctionType.Identity)
                    nc.scalar.activation(hn[:], s2[:], func, bias=b_t[layer][:])
                    h_cur[t] = hn

            # ---------- final AG of h4 (node-major) for the head
            h4_in = dp.tile([SHARD, P], bf16, tag="ag_in")
            h4_tab = dp.tile([NPAD, P], bf16, tag="ag_out")
            for t in range(TPC):
                tp4 = pst.tile([P, P], bf16, tag="mmt")
                nc.tensor.transpose(out=tp4[:], in_=h_cur[t][:], identity=id_t[:])
                hn4 = wk.tile([P, P], bf16, tag="hwn")
                nc.vector.tensor_copy(out=hn4[:], in_=tp4[:])
                nc.sync.dma_start(out=h4_in[t * P : (t + 1) * P, :], in_=hn4[:])
            nc.gpsimd.collective_compute(
                "AllGather",
                mybir.AluOpType.bypass,
                replica_groups=[list(range(NC))],
                ins=[h4_in[:].opt()],
                outs=[h4_tab[:].opt()],
            )

            # ---------- head MLP over pair tiles
            hgat = {}

            def get_hgather(which, call):
                key = (which, call)
                if key in hgat:
                    return hgat[key]
                g = gpool.tile([P, 1, NIDXH], bf16, tag="hg")
                cls = pl.hcall_cls[call]
                hi_sel = (cls >> 1) if which == 0 else (cls & 1)
                base = cfg.HI_BASE if hi_sel else 0
                win = cfg.WHI if hi_sel else cfg.WLO
                it = hi0_t if which == 0 else hi1_t
                w = NIDXH // 16
                nc.gpsimd.dma_gather(
                    g[:],
                    h4_tab[base : base + win, :],
                    it[:, call * w : (call + 1) * w],
                    NIDXH, nidxh_reg, F,
                    transpose=True,
                    queue_num=(call % 4),
                )
                hgat[key] = g
                return g

            for pt in range(pl.PT):
                call = pt * P // NIDXH
                sl = (pt * P % NIDXH)
                g0 = get_hgather(0, call)
                g1 = get_hgather(1, call)
                z1p = pp.tile([P, P], f32, tag="mm")
                nc.tensor.matmul(out=z1p[:], lhsT=wl1_t[:, 0:P],
                                 rhs=g0[:, 0, sl : sl + P], start=True, stop=False)
                nc.tensor.matmul(out=z1p[:], lhsT=wl1_t[:, P : 2 * P],
                                 rhs=g1[:, 0, sl : sl + P], start=False, stop=True)
                z1 = wk.tile([P, P], bf16, tag="z1")
                nc.scalar.activation(z1[:], z1p[:], mybir.ActivationFunctionType.Relu,
                                     bias=bl1_t[:])
                z2p = ppr.tile([1, P], f32, tag="row")
                nc.tensor.matmul(out=z2p[:], lhsT=wl2_t[:], rhs=z1[:],
                                 start=True, stop=True)
                zrow = zp.tile([1, P], f32, tag="z")
                nc.vector.tensor_tensor(out=zrow[:], in0=z2p[:],
                                        in1=bl2_t[:].to_broadcast([1, P]),
                                        op=mybir.AluOpType.add)
                nc.sync.dma_start(out=zout[0:1, pt * P : (pt + 1) * P], in_=zrow[:])
    nc.compile()
    return nc


# ----------------------------------------------------------------------------
# Host wrapper


def _prep_inputs(cfg, pl, x, weights, core):
    (W0, b0, W1, b1, W2, b2, W3, b3, Wl1, bl1, Wl2, bl2) = weights
    SHARD = cfg.SHARD
    xp = np.zeros((cfg.NPAD, cfg.F), dtype=np.float32)
    xp[: x.shape[0]] = x
    xT = xp[core * SHARD : (core + 1) * SHARD].T.astype(BF16)
    m = {
        "xT": np.ascontiguousarray(xT),
        "W0": W0.astype(BF16), "W1": W1.astype(BF16),
        "W2": W2.astype(BF16), "W3": W3.astype(BF16),
        "b0": b0.reshape(-1, 1).astype(np.float32),
        "b1": b1.reshape(-1, 1).astype(np.float32),
        "b2": b2.reshape(-1, 1).astype(np.float32),
        "b3": b3.reshape(-1, 1).astype(np.float32),
        "Wl1": Wl1.astype(BF16),
        "bl1": bl1.reshape(-1, 1).astype(np.float32),
        "Wl2": Wl2.reshape(-1, 1).astype(BF16),
        "bl2": bl2.reshape(1, 1).astype(np.float32),
        "onehot": pl.onehot[core],
        "gidx_lo": pl.gidx_lo[core],
        "gidx_hi": pl.gidx_hi[core],
        "id128": np.eye(cfg.F, dtype=np.float32).astype(BF16),
        "hidx0": pl.hidx0[core],
        "hidx1": pl.hidx1[core],
    }
    return m


def _unpack_head(cfg, pl, zouts):
    """Per-core zout [1, HTOT] -> global z [NP] in original pair order."""
    zs = []
    for c in range(cfg.NC):
        z = zouts[c].reshape(-1)
        parts = []
        off = 0
        for k in range(4):
            n = int(pl.hcounts[c, k])
            parts.append(z[off : off + n])
            off += pl.CLS_PAD[k]
        zc = np.concatenate(parts)
        orig = np.empty(pl.PPC, dtype=np.float32)
        orig[pl.hperm[c]] = zc
        zs.append(orig)
    return np.concatenate(zs)


def run(cfg, x, edge_index, pairs, weights, trace=False):
    pl = make_plan(cfg, edge_index, pairs)
    nc = build_bass(cfg, pl)
    in_maps = [_prep_inputs(cfg, pl, x, weights, c) for c in range(cfg.NC)]
    res = run_bass_kernel_spmd(nc, in_maps, core_ids=list(range(cfg.NC)), trace=trace)
    z = _unpack_head(cfg, pl, [res.results[c]["zout"] for c in range(cfg.NC)])
    return z, res


def kernel(x, edge_index, pos_edges_train, neg_edges_train, pos_edges_test,
           neg_edges_test, W0, b0, W1, b1, W2, b2, W3, b3, Wl1, bl1, Wl2, bl2):
    cfg = full_cfg()
    pairs = np.concatenate([
        np.asarray(pos_edges_train).T, np.asarray(neg_edges_train).T,
        np.asarray(pos_edges_test).T, np.asarray(neg_edges_test).T,
    ], axis=0)
    weights = (np.asarray(W0), np.asarray(b0), np.asarray(W1), np.asarray(b1),
               np.asarray(W2), np.asarray(b2), np.asarray(W3), np.asarray(b3),
               np.asarray(Wl1), np.asarray(bl1), np.asarray(Wl2), np.asarray(bl2))
    trace = bool(int(os.environ.get("GCN_TRACE", "0")))
    z, res = run(cfg, np.asarray(x), np.asarray(edge_index), pairs, weights,
                 trace=trace)
    global LAST_EXEC_NS
    LAST_EXEC_NS = res.exec_time_ns
    n_train = pos_edges_train.shape[1] + neg_edges_train.shape[1]
    return z[:n_train].astype(np.float32), z[n_train:].astype(np.float32)


# revision 18
# speedup vs baseline: 1.1473x; 1.0739x over previous
"""GCN edge-prediction kernel for 8 Trainium2 NeuronCores.

Strategy (per sharding hint): nodes (and segment_sum outputs) are sharded
contiguously across the 8 cores; each GCN layer all-gathers the
degree-scaled transformed features (the halo exchange, which for a random
graph is everything), then each core gathers its own edges' source rows via
batched SWDGE dma_gather and scatter-adds them with one-hot selector
matmuls on the tensor engine. The edge-pair MLP head is data-parallel over
edge pairs. Small weight matrices are replicated.
"""

import os
import sys
import types

import numpy as np
import ml_dtypes

import concourse.bacc as bacc
import concourse.bass as bass
import concourse.mybir as mybir
import concourse.tile as tile
from concourse.vector_clock import ScopedClock
from concourse.bass_utils import run_bass_kernel_spmd
from concourse.library_config import mlp as _mlp_library

BF16 = ml_dtypes.bfloat16
LAST_EXEC_NS = None

# ----------------------------------------------------------------------------
# Workaround: walrus rejects instructions with more than a few sem waits; the
# TileContext tail drain accumulates one wait per logical processor. Split
# them across preceding sync-engine nops (1 wait each).


def _patched_drain_and_barrier(self, tick_clock, wait_clock):
    nops = [self.nc.sync.nop(nofuse=True) for _ in range(30)]
    drain_inst = self.nc.sync.drain()
    wait_clock.add_sem_waits(
        drain_inst.ins, ScopedClock({None: tick_clock.global_clock})
    )
    si = drain_inst.ins.sync_info
    waits = list(si.on_wait) if si and si.on_wait else []
    if waits:
        chunks = [waits[i : i + 1] for i in range(0, len(waits), 1)]
        assert len(chunks) <= len(nops), f"too many wait chunks: {len(chunks)}"
        for nop_inst, chunk in zip(nops, chunks):
            nsi = nop_inst.ins.sync_info
            if nsi is None:
                nop_inst.ins.sync_info = mybir.SyncInfo(on_wait=chunk, on_update=[])
            else:
                nsi.on_wait = chunk
        si.on_wait = []
    self.nc.all_engine_barrier()
    popped = self.nc._tile_sem_poison_stack.pop()
    assert popped is self._sem_poison
    self.nc.clear_and_free_semaphores(list(self.sems.allocated().values()))
    self.nc.all_engine_barrier()


tile.TileContext._drain_and_barrier = _patched_drain_and_barrier

# ----------------------------------------------------------------------------
# Workaround 2: Tile assigns SWDGE completion-sem lanes round-robin without
# regard to the SWDGE queue an instruction targets, but a lane's semaphore is
# locked to one queue. Pin dma_gather lanes to their queue_num (lanes 0-3) and
# keep other Pool DMAs on lanes 4-7.

from concourse import tile_sem_assignment as _tsa

_orig_assign_tick = _tsa.TileClockTick._assign_tick


def _patched_assign_tick(self, inst):
    if (
        isinstance(inst, _tsa.DMAInst)
        and inst.engine == mybir.EngineType.Pool
        and not isinstance(inst, _tsa.bass_isa.UserSyncedRemoteDMADescs)
    ):
        if isinstance(inst, mybir.InstDMAGatherAnt):
            self.next_sw_dma_idx = (getattr(inst, "queue_num", 0) or 0) % 4
        else:
            alt = getattr(self, "_np_alt", 0)
            self._np_alt = alt + 1
            self.next_sw_dma_idx = 4 + alt % 4
    return _orig_assign_tick(self, inst)


_tsa.TileClockTick._assign_tick = _patched_assign_tick

# ----------------------------------------------------------------------------
# Configuration

P = 128


class Cfg:
    def __init__(self, n_nodes, n_cores, tiles_per_core, f, nidx, hi_base, lo_lim,
                 nidxh=None):
        self.N = n_nodes
        self.NC = n_cores
        self.TPC = tiles_per_core
        self.SHARD = tiles_per_core * P
        self.NPAD = self.SHARD * n_cores
        self.F = f
        self.NIDX = nidx
        self.NIDXH = nidxh or nidx
        self.HI_BASE = hi_base  # base row offset of the "hi" gather window
        self.LO_LIM = lo_lim    # node ids < LO_LIM use the lo window
        self.WLO = min(32768, self.NPAD)
        self.WHI = self.NPAD - hi_base
        assert self.NPAD >= n_nodes
        assert lo_lim <= self.WLO
        assert self.WHI <= 32768


def full_cfg():
    return Cfg(n_nodes=50000, n_cores=8, tiles_per_core=49, f=128, nidx=1024,
               hi_base=50176 - 32768, lo_lim=32768, nidxh=512)


# ----------------------------------------------------------------------------
# Host-side planning: uniform SPMD structure + per-core index/selector data


def _ceil_to(x, m):
    return (x + m - 1) // m * m


class Plan:
    pass


def _balance_perm(dst_old, cfg):
    """Node renumbering balancing per-tile in-degree (LPT bin packing).
    Returns ren: old_id -> new_id over [0, NPAD)."""
    import heapq

    NPAD = cfg.NPAD
    NTILES = NPAD // P
    indeg = np.bincount(np.asarray(dst_old, np.int64), minlength=NPAD)
    order = np.argsort(-indeg, kind="stable")
    heap = [(0, t) for t in range(NTILES)]
    heapq.heapify(heap)
    fill = np.zeros(NTILES, np.int64)
    ren = np.empty(NPAD, np.int64)
    for old in order:
        load, t = heapq.heappop(heap)
        ren[old] = t * P + fill[t]
        fill[t] += 1
        if fill[t] < P:
            heapq.heappush(heap, (load + int(indeg[old]), t))
    return ren


def make_plan(cfg, edge_index, pairs):
    """edge_index: [2, E] int; pairs: [NP, 2] int (edge-MLP endpoint pairs)."""
    pl = Plan()
    NC, TPC, SHARD, NIDX = cfg.NC, cfg.TPC, cfg.SHARD, cfg.NIDX
    src_old = np.asarray(edge_index[0], dtype=np.int64)
    dst_old = np.asarray(edge_index[1], dtype=np.int64)
    ren = _balance_perm(dst_old, cfg)
    pl.ren = ren
    src = ren[src_old]
    dst = ren[dst_old]
    # class split: forced lo if s < HI_BASE, forced hi if s >= WLO,
    # flexible in the overlap [HI_BASE, WLO) -> used to balance lo/hi per tile
    core_of = dst // SHARD
    per_core = []
    for c in range(NC):
        m = core_of == c
        s, d = src[m], dst[m]
        tloc = (d - c * SHARD) // P
        nloc = (d - c * SHARD) % P
        hi = np.zeros(len(s), np.int64)
        for t in range(TPC):
            tm = np.nonzero(tloc == t)[0]
            st = s[tm]
            forced_hi = st >= cfg.WLO
            flex = (st >= cfg.HI_BASE) & ~forced_hi
            n_tot = len(tm)
            n_lo_f = int(np.sum(~forced_hi & ~flex))
            n_flex = int(np.sum(flex))
            lo_lo = max(n_lo_f, n_tot - 1024)
            lo_hi_ = min(1024, n_lo_f + n_flex)
            lo_t = min(max(n_tot // 2, lo_lo), lo_hi_) if lo_lo <= lo_hi_ else n_lo_f + n_flex
            k = lo_t - n_lo_f  # flex edges assigned to lo
            hi_t = np.asarray(forced_hi, np.int64)
            flex_idx = np.nonzero(flex)[0]
            hi_t[flex_idx[max(0, k):]] = 1
            hi[tm] = hi_t
        order = np.lexsort((s, hi, tloc))
        per_core.append((s[order], tloc[order], nloc[order], hi[order]))

    # uniform per-(tile,class) block counts
    def blkmax(cls):
        mx = 1
        for c in range(NC):
            s, tloc, nloc, hi = per_core[c]
            for t in range(TPC):
                n = int(np.sum((tloc == t) & (hi == cls)))
                mx = max(mx, (n + P - 1) // P)
        return mx

    BLO, BHI = blkmax(0), blkmax(1)
    pl.BLO, pl.BHI = BLO, BHI
    pl.NBLK = TPC * (BLO + BHI)
    stream_lo = TPC * BLO * P
    stream_hi = TPC * BHI * P
    pl.CALLS_LO = (stream_lo + NIDX - 1) // NIDX
    pl.CALLS_HI = (stream_hi + NIDX - 1) // NIDX

    pl.gidx_lo = []
    pl.gidx_hi = []
    pl.onehot = []
    for c in range(NC):
        s, tloc, nloc, hi = per_core[c]
        idx_lo = np.zeros(pl.CALLS_LO * NIDX, dtype=np.int64)
        idx_hi = np.zeros(pl.CALLS_HI * NIDX, dtype=np.int64)
        oh = np.zeros((pl.NBLK * P, P), dtype=np.float32)
        for t in range(TPC):
            for cls in (0, 1):
                m = (tloc == t) & (hi == cls)
                ss, nn = s[m], nloc[m]
                k = np.arange(len(ss))
                if cls == 0:
                    idx_lo[t * BLO * P + k] = ss
                    blk = t * (BLO + BHI) + k // P
                else:
                    idx_hi[t * BHI * P + k] = ss - cfg.HI_BASE
                    blk = t * (BLO + BHI) + BLO + k // P
                oh[blk * P + k % P, nn] = 1.0
        pl.gidx_lo.append(_wrap_idx(idx_lo, NIDX))
        pl.gidx_hi.append(_wrap_idx(idx_hi, NIDX))
        pl.onehot.append(oh.astype(BF16))

    # ---- head: pair classes by (e0 hi, e1 hi), padded to uniform sizes
    NP_ = pairs.shape[0]
    assert NP_ % NC == 0
    PPC = NP_ // NC
    pl.PPC = PPC
    e0 = ren[np.asarray(pairs[:, 0], dtype=np.int64)].reshape(NC, PPC)
    e1 = ren[np.asarray(pairs[:, 1], dtype=np.int64)].reshape(NC, PPC)

    # class of a pair: 2*(e0 in hi window) + (e1 in hi window); endpoints in
    # the overlap [HI_BASE, WLO) may use either window -> greedy balance
    def ep_opts(v):
        if v < cfg.HI_BASE:
            return (0,)
        if v >= cfg.WLO:
            return (1,)
        return (0, 1)

    cls_all = np.zeros((NC, PPC), np.int64)
    counts = np.zeros((NC, 4), dtype=np.int64)
    for c in range(NC):
        cnt = [0, 0, 0, 0]
        e0c, e1c = e0[c], e1[c]
        for i in range(PPC):
            opts = [2 * a + b for a in ep_opts(int(e0c[i])) for b in ep_opts(int(e1c[i]))]
            k = min(opts, key=lambda o: cnt[o])
            cls_all[c, i] = k
            cnt[k] += 1
        counts[c] = cnt
    NIDXH = cfg.NIDXH
    pl.CLS_PAD = [_ceil_to(int(counts[:, k].max()), NIDXH) for k in range(4)]
    pl.HTOT = int(sum(pl.CLS_PAD))
    pl.PT = pl.HTOT // P
    pl.HCALLS = pl.HTOT // NIDXH
    # per-call (base0, base1) selection: class k -> e0 base = k >> 1, e1 = k & 1
    pl.hcall_cls = []
    for k in range(4):
        pl.hcall_cls += [k] * (pl.CLS_PAD[k] // NIDXH)
    pl.hidx0 = []
    pl.hidx1 = []
    pl.hperm = []
    pl.hcounts = counts
    for c in range(NC):
        i0 = np.zeros(pl.HTOT, dtype=np.int64)
        i1 = np.zeros(pl.HTOT, dtype=np.int64)
        order = np.argsort(cls_all[c], kind="stable")
        pl.hperm.append(order)
        off = 0
        pos = 0
        for k in range(4):
            n = int(counts[c, k])
            sel = order[pos : pos + n]
            a0 = e0[c][sel] - (cfg.HI_BASE if k >= 2 else 0)
            a1 = e1[c][sel] - (cfg.HI_BASE if (k & 1) else 0)
            i0[off : off + n] = a0
            i1[off : off + n] = a1
            off += pl.CLS_PAD[k]
            pos += n
        pl.hidx0.append(_wrap_idx(i0, NIDXH))
        pl.hidx1.append(_wrap_idx(i1, NIDXH))
    return pl


def _wrap_idx(flat, nidx):
    """[L] -> [128, (L/nidx)*(nidx/16)] int16 in dma_gather's wrapped layout."""
    assert len(flat) % nidx == 0
    ncall = len(flat) // nidx
    w = nidx // 16
    out = np.zeros((16, ncall * w), dtype=np.int16)
    for b in range(ncall):
        out[:, b * w : (b + 1) * w] = flat[b * nidx : (b + 1) * nidx].reshape(w, 16).T
    return np.tile(out, (8, 1))


# ----------------------------------------------------------------------------
# Bass program


def build_bass(cfg, pl, trace_friendly=False):
    NC, TPC, F, NIDX = cfg.NC, cfg.TPC, cfg.F, cfg.NIDX
    NPAD = cfg.NPAD
    SHARD = cfg.SHARD
    BLO, BHI = pl.BLO, pl.BHI
    NBT = BLO + BHI
    f32 = mybir.dt.float32
    bf16 = mybir.dt.bfloat16
    i16 = mybir.dt.int16

    nc = bacc.Bacc("TRN2", num_swdge_queues=4, dynamic_dma_scratch_size=32768)

    # ---- dram inputs
    xT = nc.dram_tensor("xT", [P, SHARD], bf16, kind="ExternalInput")
    Ws = [nc.dram_tensor(f"W{i}", [P, P], bf16, kind="ExternalInput") for i in range(4)]
    bs = [nc.dram_tensor(f"b{i}", [P, 1], f32, kind="ExternalInput") for i in range(4)]
    Wl1 = nc.dram_tensor("Wl1", [2 * P, P], bf16, kind="ExternalInput")
    bl1 = nc.dram_tensor("bl1", [P, 1], f32, kind="ExternalInput")
    Wl2 = nc.dram_tensor("Wl2", [P, 1], bf16, kind="ExternalInput")
    bl2 = nc.dram_tensor("bl2", [1, 1], f32, kind="ExternalInput")
    oh_d = nc.dram_tensor("onehot", [pl.NBLK * P, P], bf16, kind="ExternalInput")
    id_d = nc.dram_tensor("id128", [P, P], bf16, kind="ExternalInput")
    gil_d = nc.dram_tensor("gidx_lo", [P, pl.CALLS_LO * (NIDX // 16)], i16,
                           kind="ExternalInput")
    gih_d = nc.dram_tensor("gidx_hi", [P, pl.CALLS_HI * (NIDX // 16)], i16,
                           kind="ExternalInput")
    NIDXH = cfg.NIDXH
    hi0_d = nc.dram_tensor("hidx0", [P, pl.HCALLS * (NIDXH // 16)], i16,
                           kind="ExternalInput")
    hi1_d = nc.dram_tensor("hidx1", [P, pl.HCALLS * (NIDXH // 16)], i16,
                           kind="ExternalInput")
    zout = nc.dram_tensor("zout", [1, pl.HTOT], f32, kind="ExternalOutput")

    oh_v = oh_d.ap().rearrange("(b p) n -> p b n", p=P)  # [128, NBLK, 128]

    with tile.TileContext(nc) as tc:
        nc.gpsimd.load_library(_mlp_library)
        nidx_reg = nc.gpsimd.to_reg(NIDX)
        nidxh_reg = nc.gpsimd.to_reg(cfg.NIDXH)
        with (
            tc.tile_pool(name="resident", bufs=1) as rp,
            tc.tile_pool(name="hA", bufs=TPC) as hA,
            tc.tile_pool(name="hB", bufs=TPC) as hB,
            tc.tile_pool(name="hwp", bufs=TPC) as hwp,
            tc.tile_pool(name="oh", bufs=3) as ohp,
            tc.tile_pool(name="g", bufs=6) as gpool,
            tc.tile_pool(name="work", bufs=4) as wk,
            tc.tile_pool(name="zp", bufs=8) as zp,
            tc.tile_pool(name="psum", bufs=3, space="PSUM") as pp,
            tc.tile_pool(name="pst", bufs=2, space="PSUM") as pst,
            tc.tile_pool(name="psrow", bufs=2, space="PSUM") as ppr,
            tc.tile_pool(name="dram", bufs=2, space="DRAM") as dp,
        ):
            # ---------- load resident tensors
            w_t = []
            for i in range(4):
                w = rp.tile([P, P], bf16, tag=f"W{i}")
                nc.sync.dma_start(out=w[:], in_=Ws[i][:, :])
                w_t.append(w)
            b_t = []
            for i in range(4):
                b = rp.tile([P, 1], f32, tag=f"b{i}")
                nc.sync.dma_start(out=b[:], in_=bs[i][:, :])
                b_t.append(b)
            wl1_t = rp.tile([P, 2 * P], bf16, tag="Wl1")
            nc.sync.dma_start(
                out=wl1_t[:].rearrange("p (k q) -> p k q", k=2),
                in_=Wl1.ap().rearrange("(k p) q -> p k q", p=P),
            )
            bl1_t = rp.tile([P, 1], f32, tag="bl1")
            nc.sync.dma_start(out=bl1_t[:], in_=bl1[:, :])
            wl2_t = rp.tile([P, 1], bf16, tag="Wl2")
            nc.sync.dma_start(out=wl2_t[:], in_=Wl2[:, :])
            bl2_t = rp.tile([1, 1], f32, tag="bl2")
            nc.sync.dma_start(out=bl2_t[:], in_=bl2[:, :])
            gil_t = rp.tile([P, pl.CALLS_LO * (NIDX // 16)], i16, tag="gil")
            nc.sync.dma_start(out=gil_t[:], in_=gil_d[:, :])
            gih_t = rp.tile([P, pl.CALLS_HI * (NIDX // 16)], i16, tag="gih")
            nc.sync.dma_start(out=gih_t[:], in_=gih_d[:, :])
            hi0_t = rp.tile([P, pl.HCALLS * (NIDXH // 16)], i16, tag="hi0")
            nc.sync.dma_start(out=hi0_t[:], in_=hi0_d[:, :])
            hi1_t = rp.tile([P, pl.HCALLS * (NIDXH // 16)], i16, tag="hi1")
            nc.sync.dma_start(out=hi1_t[:], in_=hi1_d[:, :])
            ones_e = rp.tile([P, 1], bf16, tag="ones_e")
            nc.gpsimd.memset(ones_e[:], 1.0)
            ones_k1 = rp.tile([1, P], f32, tag="ones_k1")
            nc.gpsimd.memset(ones_k1[:], 1.0)
            id_t = rp.tile([P, P], bf16, tag="id128")
            nc.sync.dma_start(out=id_t[:], in_=id_d[:, :])

            # initial h (own shard, feature-major)
            h_cur = []
            for t in range(TPC):
                ht = hA.tile([P, P], bf16, tag="h")
                nc.sync.dma_start(out=ht[:], in_=xT[:, t * P : (t + 1) * P])
                h_cur.append(ht)

            # ---------- degree pass: deg[node] = sum_e onehot[e, node]
            deg_sb = rp.tile([1, SHARD], f32, tag="deg")
            for t in range(TPC):
                oh_t = ohp.tile([P, NBT * P], bf16, tag="oh")
                nc.sync.dma_start(
                    out=oh_t[:].rearrange("p (b n) -> p b n", n=P),
                    in_=oh_v[:, t * NBT : (t + 1) * NBT, :],
                )
                dps = ppr.tile([1, P], f32, tag="row")
                for j in range(NBT):
                    nc.tensor.matmul(
                        out=dps[:],
                        lhsT=ones_e[:],
                        rhs=oh_t[:, j * P : (j + 1) * P],
                        start=(j == 0),
                        stop=(j == NBT - 1),
                    )
                nc.vector.tensor_copy(out=deg_sb[0:1, t * P : (t + 1) * P], in_=dps[:])
            # dis = sqrt(1/(deg+1)) broadcast to all partitions (in-place chain)
            nc.vector.tensor_scalar_add(deg_sb[:], deg_sb[:], 1.0)
            nc.vector.reciprocal(deg_sb[:], deg_sb[:])
            nc.scalar.activation(deg_sb[:], deg_sb[:], mybir.ActivationFunctionType.Sqrt)
            dis_bc = rp.tile([P, SHARD], f32, tag="dis_bc")
            for o in range(0, SHARD, 512):
                w = min(512, SHARD - o)
                bps = pp.tile([P, 512], f32, tag="mm")
                nc.tensor.matmul(out=bps[:, :w], lhsT=ones_k1[:],
                                 rhs=deg_sb[0:1, o : o + w], start=True, stop=True)
                nc.vector.tensor_copy(out=dis_bc[:, o : o + w], in_=bps[:, :w])

            # ---------- GCN layers
            hpools = [hA, hB]
            for layer in range(4):
                # phase A: hw' = (h @ W) * dis, transpose to node-major, stage AG in
                ag_in = dp.tile([SHARD, P], bf16, tag="ag_in")
                ag_out = dp.tile([NPAD, P], bf16, tag="ag_out")
                hw_tiles = []
                for t in range(TPC):
                    mm = pp.tile([P, P], f32, tag="mm")
                    nc.tensor.matmul(out=mm[:], lhsT=w_t[layer][:], rhs=h_cur[t][:],
                                     start=True, stop=True)
                    hw = hwp.tile([P, P], bf16, tag="hw")
                    nc.vector.tensor_tensor(
                        out=hw[:], in0=mm[:], in1=dis_bc[:, t * P : (t + 1) * P],
                        op=mybir.AluOpType.mult,
                    )
                    hw_tiles.append(hw)
                    tp = pst.tile([P, P], bf16, tag="mmt")
                    nc.tensor.transpose(out=tp[:], in_=hw[:], identity=id_t[:])
                    hwn = wk.tile([P, P], bf16, tag="hwn")
                    nc.vector.tensor_copy(out=hwn[:], in_=tp[:])
                    nc.sync.dma_start(out=ag_in[t * P : (t + 1) * P, :], in_=hwn[:])
                nc.gpsimd.collective_compute(
                    "AllGather",
                    mybir.AluOpType.bypass,
                    replica_groups=[list(range(NC))],
                    ins=[ag_in[:].opt()],
                    outs=[ag_out[:].opt()],
                )

                # phase C: gather + scatter-matmul + epilogue
                gat = {}

                def get_gather(stream, call):
                    key = (layer, stream, call)
                    if key in gat:
                        return gat[key]
                    g = gpool.tile([P, NIDX // P, F], bf16, tag="g")
                    if stream == 0:
                        base, win, it = 0, cfg.WLO, gil_t
                    else:
                        base, win, it = cfg.HI_BASE, cfg.WHI, gih_t
                    w = NIDX // 16
                    nc.gpsimd.dma_gather(
                        g[:],
                        ag_out[base : base + win, :],
                        it[:, call * w : (call + 1) * w],
                        NIDX, nidx_reg, F,
                        queue_num=(call % 4),
                    )
                    gat[key] = g
                    return g

                for t in range(TPC):
                    oh_t = ohp.tile([P, NBT * P], bf16, tag="oh")
                    nc.sync.dma_start(
                        out=oh_t[:].rearrange("p (b n) -> p b n", n=P),
                        in_=oh_v[:, t * NBT : (t + 1) * NBT, :],
                    )
                    agg = pp.tile([P, P], f32, tag="mm")
                    nb = 0
                    for cls in (0, 1):
                        BU = BLO if cls == 0 else BHI
                        for j in range(BU):
                            pos = (t * BU + j) * P
                            g = get_gather(cls, pos // NIDX)
                            sl = (pos % NIDX) // P
                            nc.tensor.matmul(
                                out=agg[:],
                                lhsT=g[:, sl, :],
                                rhs=oh_t[:, (cls * BLO + j) * P : (cls * BLO + j + 1) * P],
                                start=(nb == 0),
                                stop=(nb == NBT - 1),
                            )
                            nb += 1
                    # epilogue: h' = act(dis * (agg + hw') + b)
                    s1 = wk.tile([P, P], f32, tag="s1")
                    nc.vector.tensor_tensor(out=s1[:], in0=agg[:], in1=hw_tiles[t][:],
                                            op=mybir.AluOpType.add)
                    s2 = wk.tile([P, P], f32, tag="s2")
                    nc.vector.tensor_tensor(out=s2[:], in0=s1[:],
                                            in1=dis_bc[:, t * P : (t + 1) * P],
                                            op=mybir.AluOpType.mult)
                    hn = hpools[(layer + 1) % 2].tile([P, P], bf16, tag="h")
                    func = (mybir.ActivationFunctionType.Relu if layer < 3
                            else mybir.ActivationFunctionType.Identity)
                    nc.scalar.activation(hn[:], s2[:], func, bias=b_t[layer][:])
                    h_cur[t] = hn

            # ---------- final AG of h4 (node-major) for the head
            h4_in = dp.tile([SHARD, P], bf16, tag="ag_in")
            h4_tab = dp.tile([NPAD, P], bf16, tag="ag_out")
            for t in range(TPC):
                tp4 = pst.tile([P, P], bf16, tag="mmt")
                nc.tensor.transpose(out=tp4[:], in_=h_cur[t][:], identity=id_t[:])
                hn4 = wk.tile([P, P], bf16, tag="hwn")
                nc.vector.tensor_copy(out=hn4[:], in_=tp4[:])
                nc.sync.dma_start(out=h4_in[t * P : (t + 1) * P, :], in_=hn4[:])
            nc.gpsimd.collective_compute(
                "AllGather",
                mybir.AluOpType.bypass,
                replica_groups=[list(range(NC))],
                ins=[h4_in[:].opt()],
                outs=[h4_tab[:].opt()],
            )

            # ---------- head MLP over pair tiles
            hgat = {}

            def get_hgather(which, call):
                key = (which, call)
                if key in hgat:
                    return hgat[key]
                g = gpool.tile([P, 1, NIDXH], bf16, tag="hg")
                cls = pl.hcall_cls[call]
                hi_sel = (cls >> 1) if which == 0 else (cls & 1)
                base = cfg.HI_BASE if hi_sel else 0
                win = cfg.WHI if hi_sel else cfg.WLO
                it = hi0_t if which == 0 else hi1_t
                w = NIDXH // 16
                nc.gpsimd.dma_gather(
                    g[:],
                    h4_tab[base : base + win, :],
                    it[:, call * w : (call + 1) * w],
                    NIDXH, nidxh_reg, F,
                    transpose=True,
                    queue_num=(call % 4),
                )
                hgat[key] = g
                return g

            for pt in range(pl.PT):
                call = pt * P // NIDXH
                sl = (pt * P % NIDXH)
                g0 = get_hgather(0, call)
                g1 = get_hgather(1, call)
                z1p = pp.tile([P, P], f32, tag="mm")
                nc.tensor.matmul(out=z1p[:], lhsT=wl1_t[:, 0:P],
                                 rhs=g0[:, 0, sl : sl + P], start=True, stop=False)
                nc.tensor.matmul(out=z1p[:], lhsT=wl1_t[:, P : 2 * P],
                                 rhs=g1[:, 0, sl : sl + P], start=False, stop=True)
                z1 = wk.tile([P, P], bf16, tag="z1")
                nc.scalar.activation(z1[:], z1p[:], mybir.ActivationFunctionType.Relu,
                                     bias=bl1_t[:])
                z2p = ppr.tile([1, P], f32, tag="row")
                nc.tensor.matmul(out=z2p[:], lhsT=wl2_t[:], rhs=z1[:],
                                 start=True, stop=True)
                zrow = zp.tile([1, P], f32, tag="z")
                nc.vector.tensor_tensor(out=zrow[:], in0=z2p[:],
                                        in1=bl2_t[:].to_broadcast([1, P]),
                                        op=mybir.AluOpType.add)
                nc.sync.dma_start(out=zout[0:1, pt * P : (pt + 1) * P], in_=zrow[:])
    nc.compile()
    return nc


# ----------------------------------------------------------------------------
# Host wrapper


def _prep_inputs(cfg, pl, x, weights, core):
    (W0, b0, W1, b1, W2, b2, W3, b3, Wl1, bl1, Wl2, bl2) = weights
    SHARD = cfg.SHARD
    xp = np.zeros((cfg.NPAD, cfg.F), dtype=np.float32)
    xp[pl.ren[: x.shape[0]]] = x
    xT = xp[core * SHARD : (core + 1) * SHARD].T.astype(BF16)
    m = {
        "xT": np.ascontiguousarray(xT),
        "W0": W0.astype(BF16), "W1": W1.astype(BF16),
        "W2": W2.astype(BF16), "W3": W3.astype(BF16),
        "b0": b0.reshape(-1, 1).astype(np.float32),
        "b1": b1.reshape(-1, 1).astype(np.float32),
        "b2": b2.reshape(-1, 1).astype(np.float32),
        "b3": b3.reshape(-1, 1).astype(np.float32),
        "Wl1": Wl1.astype(BF16),
        "bl1": bl1.reshape(-1, 1).astype(np.float32),
        "Wl2": Wl2.reshape(-1, 1).astype(BF16),
        "bl2": bl2.reshape(1, 1).astype(np.float32),
        "onehot": pl.onehot[core],
        "gidx_lo": pl.gidx_lo[core],
        "gidx_hi": pl.gidx_hi[core],
        "id128": np.eye(cfg.F, dtype=np.float32).astype(BF16),
        "hidx0": pl.hidx0[core],
        "hidx1": pl.hidx1[core],
    }
    return m


def _unpack_head(cfg, pl, zouts):
    """Per-core zout [1, HTOT] -> global z [NP] in original pair order."""
    zs = []
    for c in range(cfg.NC):
        z = zouts[c].reshape(-1)
        parts = []
        off = 0
        for k in range(4):
            n = int(pl.hcounts[c, k])
            parts.append(z[off : off + n])
            off += pl.CLS_PAD[k]
        zc = np.concatenate(parts)
        orig = np.empty(pl.PPC, dtype=np.float32)
        orig[pl.hperm[c]] = zc
        zs.append(orig)
    return np.concatenate(zs)


def run(cfg, x, edge_index, pairs, weights, trace=False):
    pl = make_plan(cfg, edge_index, pairs)
    nc = build_bass(cfg, pl)
    in_maps = [_prep_inputs(cfg, pl, x, weights, c) for c in range(cfg.NC)]
    res = run_bass_kernel_spmd(nc, in_maps, core_ids=list(range(cfg.NC)), trace=trace)
    z = _unpack_head(cfg, pl, [res.results[c]["zout"] for c in range(cfg.NC)])
    return z, res


def kernel(x, edge_index, pos_edges_train, neg_edges_train, pos_edges_test,
           neg_edges_test, W0, b0, W1, b1, W2, b2, W3, b3, Wl1, bl1, Wl2, bl2):
    cfg = full_cfg()
    pairs = np.concatenate([
        np.asarray(pos_edges_train).T, np.asarray(neg_edges_train).T,
        np.asarray(pos_edges_test).T, np.asarray(neg_edges_test).T,
    ], axis=0)
    weights = (np.asarray(W0), np.asarray(b0), np.asarray(W1), np.asarray(b1),
               np.asarray(W2), np.asarray(b2), np.asarray(W3), np.asarray(b3),
               np.asarray(Wl1), np.asarray(bl1), np.asarray(Wl2), np.asarray(bl2))
    trace = bool(int(os.environ.get("GCN_TRACE", "0")))
    z, res = run(cfg, np.asarray(x), np.asarray(edge_index), pairs, weights,
                 trace=trace)
    global LAST_EXEC_NS
    LAST_EXEC_NS = res.exec_time_ns
    n_train = pos_edges_train.shape[1] + neg_edges_train.shape[1]
    return z[:n_train].astype(np.float32), z[n_train:].astype(np.float32)


# revision 19
# speedup vs baseline: 2.0982x; 1.8288x over previous
"""GCN edge-prediction kernel for 8 Trainium2 NeuronCores.

Strategy (per sharding hint): nodes (and segment_sum outputs) are sharded
contiguously across the 8 cores; each GCN layer all-gathers the
degree-scaled transformed features (the halo exchange, which for a random
graph is everything), then each core gathers its own edges' source rows via
batched SWDGE dma_gather and scatter-adds them with one-hot selector
matmuls on the tensor engine. The edge-pair MLP head is data-parallel over
edge pairs. Small weight matrices are replicated.
"""

import os
import sys
import types

import numpy as np
import ml_dtypes

import concourse.bacc as bacc
import concourse.bass as bass
import concourse.mybir as mybir
import concourse.tile as tile
from concourse.vector_clock import ScopedClock
from concourse.bass_utils import run_bass_kernel_spmd
from concourse.library_config import mlp as _mlp_library

BF16 = ml_dtypes.bfloat16
LAST_EXEC_NS = None

# ----------------------------------------------------------------------------
# Workaround: walrus rejects instructions with more than a few sem waits; the
# TileContext tail drain accumulates one wait per logical processor. Split
# them across preceding sync-engine nops (1 wait each).


def _patched_drain_and_barrier(self, tick_clock, wait_clock):
    nops = [self.nc.sync.nop(nofuse=True) for _ in range(30)]
    drain_inst = self.nc.sync.drain()
    wait_clock.add_sem_waits(
        drain_inst.ins, ScopedClock({None: tick_clock.global_clock})
    )
    si = drain_inst.ins.sync_info
    waits = list(si.on_wait) if si and si.on_wait else []
    if waits:
        chunks = [waits[i : i + 1] for i in range(0, len(waits), 1)]
        assert len(chunks) <= len(nops), f"too many wait chunks: {len(chunks)}"
        for nop_inst, chunk in zip(nops, chunks):
            nsi = nop_inst.ins.sync_info
            if nsi is None:
                nop_inst.ins.sync_info = mybir.SyncInfo(on_wait=chunk, on_update=[])
            else:
                nsi.on_wait = chunk
        si.on_wait = []
    self.nc.all_engine_barrier()
    popped = self.nc._tile_sem_poison_stack.pop()
    assert popped is self._sem_poison
    self.nc.clear_and_free_semaphores(list(self.sems.allocated().values()))
    self.nc.all_engine_barrier()


tile.TileContext._drain_and_barrier = _patched_drain_and_barrier

# ----------------------------------------------------------------------------
# Workaround 2: Tile assigns SWDGE completion-sem lanes round-robin without
# regard to the SWDGE queue an instruction targets, but a lane's semaphore is
# locked to one queue. Pin dma_gather lanes to their queue_num (lanes 0-3) and
# keep other Pool DMAs on lanes 4-7.

from concourse import tile_sem_assignment as _tsa

_orig_assign_tick = _tsa.TileClockTick._assign_tick


def _patched_assign_tick(self, inst):
    if (
        isinstance(inst, _tsa.DMAInst)
        and inst.engine == mybir.EngineType.Pool
        and not isinstance(inst, _tsa.bass_isa.UserSyncedRemoteDMADescs)
    ):
        if isinstance(inst, mybir.InstDMAGatherAnt):
            q = (getattr(inst, "queue_num", 0) or 0) % 4
            alts = getattr(self, "_q_alt", None)
            if alts is None:
                alts = self._q_alt = [0, 0, 0, 0]
            self.next_sw_dma_idx = 2 * q + (alts[q] & 1)
            alts[q] += 1
        else:
            # non-gather Pool DMAs would collide with a gather queue's sem
            # lane; this kernel emits none, but keep them off lanes 0-7 wrap
            alt = getattr(self, "_np_alt", 0)
            self._np_alt = alt + 1
            self.next_sw_dma_idx = alt % 8
    return _orig_assign_tick(self, inst)


_tsa.TileClockTick._assign_tick = _patched_assign_tick

# ----------------------------------------------------------------------------
# Configuration

P = 128


class Cfg:
    def __init__(self, n_nodes, n_cores, tiles_per_core, f, nidx, hi_base, lo_lim,
                 nidxh=None):
        self.N = n_nodes
        self.NC = n_cores
        self.TPC = tiles_per_core
        self.SHARD = tiles_per_core * P
        self.NPAD = self.SHARD * n_cores
        self.F = f
        self.NIDX = nidx
        self.NIDXH = nidxh or nidx
        self.HI_BASE = hi_base  # base row offset of the "hi" gather window
        self.LO_LIM = lo_lim    # node ids < LO_LIM use the lo window
        self.WLO = min(32768, self.NPAD)
        self.WHI = self.NPAD - hi_base
        assert self.NPAD >= n_nodes
        assert lo_lim <= self.WLO
        assert self.WHI <= 32768


def full_cfg():
    return Cfg(n_nodes=50000, n_cores=8, tiles_per_core=49, f=128, nidx=1024,
               hi_base=50176 - 32768, lo_lim=32768, nidxh=512)


# ----------------------------------------------------------------------------
# Host-side planning: uniform SPMD structure + per-core index/selector data


def _ceil_to(x, m):
    return (x + m - 1) // m * m


class Plan:
    pass


def _balance_perm(dst_old, cfg):
    """Node renumbering balancing per-tile in-degree (LPT bin packing).
    Returns ren: old_id -> new_id over [0, NPAD)."""
    import heapq

    NPAD = cfg.NPAD
    NTILES = NPAD // P
    indeg = np.bincount(np.asarray(dst_old, np.int64), minlength=NPAD)
    order = np.argsort(-indeg, kind="stable")
    heap = [(0, t) for t in range(NTILES)]
    heapq.heapify(heap)
    fill = np.zeros(NTILES, np.int64)
    ren = np.empty(NPAD, np.int64)
    for old in order:
        load, t = heapq.heappop(heap)
        ren[old] = t * P + fill[t]
        fill[t] += 1
        if fill[t] < P:
            heapq.heappush(heap, (load + int(indeg[old]), t))
    return ren


def make_plan(cfg, edge_index, pairs):
    """edge_index: [2, E] int; pairs: [NP, 2] int (edge-MLP endpoint pairs)."""
    pl = Plan()
    NC, TPC, SHARD, NIDX = cfg.NC, cfg.TPC, cfg.SHARD, cfg.NIDX
    src_old = np.asarray(edge_index[0], dtype=np.int64)
    dst_old = np.asarray(edge_index[1], dtype=np.int64)
    ren = _balance_perm(dst_old, cfg)
    pl.ren = ren
    src = ren[src_old]
    dst = ren[dst_old]
    # class split: forced lo if s < HI_BASE, forced hi if s >= WLO,
    # flexible in the overlap [HI_BASE, WLO) -> used to balance lo/hi per tile
    core_of = dst // SHARD
    per_core = []
    for c in range(NC):
        m = core_of == c
        s, d = src[m], dst[m]
        tloc = (d - c * SHARD) // P
        nloc = (d - c * SHARD) % P
        hi = np.zeros(len(s), np.int64)
        for t in range(TPC):
            tm = np.nonzero(tloc == t)[0]
            st = s[tm]
            forced_hi = st >= cfg.WLO
            flex = (st >= cfg.HI_BASE) & ~forced_hi
            n_tot = len(tm)
            n_lo_f = int(np.sum(~forced_hi & ~flex))
            n_flex = int(np.sum(flex))
            lo_lo = max(n_lo_f, n_tot - 1024)
            lo_hi_ = min(1024, n_lo_f + n_flex)
            lo_t = min(max(n_tot // 2, lo_lo), lo_hi_) if lo_lo <= lo_hi_ else n_lo_f + n_flex
            k = lo_t - n_lo_f  # flex edges assigned to lo
            hi_t = np.asarray(forced_hi, np.int64)
            flex_idx = np.nonzero(flex)[0]
            hi_t[flex_idx[max(0, k):]] = 1
            hi[tm] = hi_t
        order = np.lexsort((s, hi, tloc))
        per_core.append((s[order], tloc[order], nloc[order], hi[order]))

    # uniform per-(tile,class) block counts
    def blkmax(cls):
        mx = 1
        for c in range(NC):
            s, tloc, nloc, hi = per_core[c]
            for t in range(TPC):
                n = int(np.sum((tloc == t) & (hi == cls)))
                mx = max(mx, (n + P - 1) // P)
        return mx

    BLO, BHI = blkmax(0), blkmax(1)
    pl.BLO, pl.BHI = BLO, BHI
    pl.NBLK = TPC * (BLO + BHI)
    stream_lo = TPC * BLO * P
    stream_hi = TPC * BHI * P
    pl.CALLS_LO = (stream_lo + NIDX - 1) // NIDX
    pl.CALLS_HI = (stream_hi + NIDX - 1) // NIDX

    pl.gidx_lo = []
    pl.gidx_hi = []
    pl.onehot = []
    for c in range(NC):
        s, tloc, nloc, hi = per_core[c]
        idx_lo = np.zeros(pl.CALLS_LO * NIDX, dtype=np.int64)
        idx_hi = np.zeros(pl.CALLS_HI * NIDX, dtype=np.int64)
        oh = np.zeros((pl.NBLK * P, P), dtype=np.float32)
        for t in range(TPC):
            for cls in (0, 1):
                m = (tloc == t) & (hi == cls)
                ss, nn = s[m], nloc[m]
                k = np.arange(len(ss))
                if cls == 0:
                    idx_lo[t * BLO * P + k] = ss
                    blk = t * (BLO + BHI) + k // P
                else:
                    idx_hi[t * BHI * P + k] = ss - cfg.HI_BASE
                    blk = t * (BLO + BHI) + BLO + k // P
                oh[blk * P + k % P, nn] = 1.0
        pl.gidx_lo.append(_wrap_idx(idx_lo, NIDX))
        pl.gidx_hi.append(_wrap_idx(idx_hi, NIDX))
        pl.onehot.append(oh.astype(BF16))

    # ---- head: pair classes by (e0 hi, e1 hi), padded to uniform sizes
    NP_ = pairs.shape[0]
    assert NP_ % NC == 0
    PPC = NP_ // NC
    pl.PPC = PPC
    e0 = ren[np.asarray(pairs[:, 0], dtype=np.int64)].reshape(NC, PPC)
    e1 = ren[np.asarray(pairs[:, 1], dtype=np.int64)].reshape(NC, PPC)

    # class of a pair: 2*(e0 in hi window) + (e1 in hi window); endpoints in
    # the overlap [HI_BASE, WLO) may use either window -> greedy balance
    def ep_opts(v):
        if v < cfg.HI_BASE:
            return (0,)
        if v >= cfg.WLO:
            return (1,)
        return (0, 1)

    cls_all = np.zeros((NC, PPC), np.int64)
    counts = np.zeros((NC, 4), dtype=np.int64)
    for c in range(NC):
        cnt = [0, 0, 0, 0]
        e0c, e1c = e0[c], e1[c]
        for i in range(PPC):
            opts = [2 * a + b for a in ep_opts(int(e0c[i])) for b in ep_opts(int(e1c[i]))]
            k = min(opts, key=lambda o: cnt[o])
            cls_all[c, i] = k
            cnt[k] += 1
        counts[c] = cnt
    NIDXH = cfg.NIDXH
    pl.CLS_PAD = [_ceil_to(int(counts[:, k].max()), NIDXH) for k in range(4)]
    pl.HTOT = int(sum(pl.CLS_PAD))
    pl.PT = pl.HTOT // P
    pl.HCALLS = pl.HTOT // NIDXH
    # per-call (base0, base1) selection: class k -> e0 base = k >> 1, e1 = k & 1
    pl.hcall_cls = []
    for k in range(4):
        pl.hcall_cls += [k] * (pl.CLS_PAD[k] // NIDXH)
    pl.hidx0 = []
    pl.hidx1 = []
    pl.hperm = []
    pl.hcounts = counts
    for c in range(NC):
        i0 = np.zeros(pl.HTOT, dtype=np.int64)
        i1 = np.zeros(pl.HTOT, dtype=np.int64)
        order = np.argsort(cls_all[c], kind="stable")
        pl.hperm.append(order)
        off = 0
        pos = 0
        for k in range(4):
            n = int(counts[c, k])
            sel = order[pos : pos + n]
            a0 = e0[c][sel] - (cfg.HI_BASE if k >= 2 else 0)
            a1 = e1[c][sel] - (cfg.HI_BASE if (k & 1) else 0)
            i0[off : off + n] = a0
            i1[off : off + n] = a1
            off += pl.CLS_PAD[k]
            pos += n
        pl.hidx0.append(_wrap_idx(i0, NIDXH))
        pl.hidx1.append(_wrap_idx(i1, NIDXH))
    return pl


def _wrap_idx(flat, nidx):
    """[L] -> [128, (L/nidx)*(nidx/16)] int16 in dma_gather's wrapped layout."""
    assert len(flat) % nidx == 0
    ncall = len(flat) // nidx
    w = nidx // 16
    out = np.zeros((16, ncall * w), dtype=np.int16)
    for b in range(ncall):
        out[:, b * w : (b + 1) * w] = flat[b * nidx : (b + 1) * nidx].reshape(w, 16).T
    return np.tile(out, (8, 1))


# ----------------------------------------------------------------------------
# Bass program


def build_bass(cfg, pl, trace_friendly=False):
    NC, TPC, F, NIDX = cfg.NC, cfg.TPC, cfg.F, cfg.NIDX
    NPAD = cfg.NPAD
    SHARD = cfg.SHARD
    BLO, BHI = pl.BLO, pl.BHI
    NBT = BLO + BHI
    f32 = mybir.dt.float32
    bf16 = mybir.dt.bfloat16
    i16 = mybir.dt.int16

    nc = bacc.Bacc("TRN2", num_swdge_queues=4, dynamic_dma_scratch_size=32768)

    # ---- dram inputs
    xT = nc.dram_tensor("xT", [P, SHARD], bf16, kind="ExternalInput")
    Ws = [nc.dram_tensor(f"W{i}", [P, P], bf16, kind="ExternalInput") for i in range(4)]
    bs = [nc.dram_tensor(f"b{i}", [P, 1], f32, kind="ExternalInput") for i in range(4)]
    Wl1 = nc.dram_tensor("Wl1", [2 * P, P], bf16, kind="ExternalInput")
    bl1 = nc.dram_tensor("bl1", [P, 1], f32, kind="ExternalInput")
    Wl2 = nc.dram_tensor("Wl2", [P, 1], bf16, kind="ExternalInput")
    bl2 = nc.dram_tensor("bl2", [1, 1], f32, kind="ExternalInput")
    oh_d = nc.dram_tensor("onehot", [pl.NBLK * P, P], bf16, kind="ExternalInput")
    id_d = nc.dram_tensor("id128", [P, P], bf16, kind="ExternalInput")
    gil_d = nc.dram_tensor("gidx_lo", [P, pl.CALLS_LO * (NIDX // 16)], i16,
                           kind="ExternalInput")
    gih_d = nc.dram_tensor("gidx_hi", [P, pl.CALLS_HI * (NIDX // 16)], i16,
                           kind="ExternalInput")
    NIDXH = cfg.NIDXH
    hi0_d = nc.dram_tensor("hidx0", [P, pl.HCALLS * (NIDXH // 16)], i16,
                           kind="ExternalInput")
    hi1_d = nc.dram_tensor("hidx1", [P, pl.HCALLS * (NIDXH // 16)], i16,
                           kind="ExternalInput")
    zout = nc.dram_tensor("zout", [1, pl.HTOT], f32, kind="ExternalOutput")

    oh_v = oh_d.ap().rearrange("(b p) n -> p b n", p=P)  # [128, NBLK, 128]

    with tile.TileContext(nc) as tc:
        nc.gpsimd.load_library(_mlp_library)
        nidx_reg = nc.gpsimd.to_reg(NIDX)
        nidxh_reg = nc.gpsimd.to_reg(cfg.NIDXH)
        with (
            tc.tile_pool(name="resident", bufs=1) as rp,
            tc.tile_pool(name="hA", bufs=TPC) as hA,
            tc.tile_pool(name="hB", bufs=TPC) as hB,
            tc.tile_pool(name="hwp", bufs=TPC) as hwp,
            tc.tile_pool(name="oh", bufs=3) as ohp,
            tc.tile_pool(name="g", bufs=10) as gpool,
            tc.tile_pool(name="work", bufs=4) as wk,
            tc.tile_pool(name="zp", bufs=8) as zp,
            tc.tile_pool(name="psum", bufs=3, space="PSUM") as pp,
            tc.tile_pool(name="pst", bufs=2, space="PSUM") as pst,
            tc.tile_pool(name="psrow", bufs=2, space="PSUM") as ppr,
            tc.tile_pool(name="dram", bufs=2, space="DRAM") as dp,
        ):
            # ---------- load resident tensors
            w_t = []
            for i in range(4):
                w = rp.tile([P, P], bf16, tag=f"W{i}")
                nc.sync.dma_start(out=w[:], in_=Ws[i][:, :])
                w_t.append(w)
            b_t = []
            for i in range(4):
                b = rp.tile([P, 1], f32, tag=f"b{i}")
                nc.sync.dma_start(out=b[:], in_=bs[i][:, :])
                b_t.append(b)
            wl1_t = rp.tile([P, 2 * P], bf16, tag="Wl1")
            nc.sync.dma_start(
                out=wl1_t[:].rearrange("p (k q) -> p k q", k=2),
                in_=Wl1.ap().rearrange("(k p) q -> p k q", p=P),
            )
            bl1_t = rp.tile([P, 1], f32, tag="bl1")
            nc.sync.dma_start(out=bl1_t[:], in_=bl1[:, :])
            wl2_t = rp.tile([P, 1], bf16, tag="Wl2")
            nc.sync.dma_start(out=wl2_t[:], in_=Wl2[:, :])
            bl2_t = rp.tile([1, 1], f32, tag="bl2")
            nc.sync.dma_start(out=bl2_t[:], in_=bl2[:, :])
            gil_t = rp.tile([P, pl.CALLS_LO * (NIDX // 16)], i16, tag="gil")
            nc.sync.dma_start(out=gil_t[:], in_=gil_d[:, :])
            gih_t = rp.tile([P, pl.CALLS_HI * (NIDX // 16)], i16, tag="gih")
            nc.sync.dma_start(out=gih_t[:], in_=gih_d[:, :])
            hi0_t = rp.tile([P, pl.HCALLS * (NIDXH // 16)], i16, tag="hi0")
            nc.sync.dma_start(out=hi0_t[:], in_=hi0_d[:, :])
            hi1_t = rp.tile([P, pl.HCALLS * (NIDXH // 16)], i16, tag="hi1")
            nc.sync.dma_start(out=hi1_t[:], in_=hi1_d[:, :])
            ones_e = rp.tile([P, 1], bf16, tag="ones_e")
            nc.gpsimd.memset(ones_e[:], 1.0)
            ones_k1 = rp.tile([1, P], f32, tag="ones_k1")
            nc.gpsimd.memset(ones_k1[:], 1.0)
            id_t = rp.tile([P, P], bf16, tag="id128")
            nc.sync.dma_start(out=id_t[:], in_=id_d[:, :])

            # initial h (own shard, feature-major)
            h_cur = []
            for t in range(TPC):
                ht = hA.tile([P, P], bf16, tag="h")
                nc.sync.dma_start(out=ht[:], in_=xT[:, t * P : (t + 1) * P])
                h_cur.append(ht)

            # ---------- degree pass: deg[node] = sum_e onehot[e, node]
            deg_sb = rp.tile([1, SHARD], f32, tag="deg")
            for t in range(TPC):
                oh_t = ohp.tile([P, NBT * P], bf16, tag="oh")
                nc.sync.dma_start(
                    out=oh_t[:].rearrange("p (b n) -> p b n", n=P),
                    in_=oh_v[:, t * NBT : (t + 1) * NBT, :],
                )
                dps = ppr.tile([1, P], f32, tag="row")
                for j in range(NBT):
                    nc.tensor.matmul(
                        out=dps[:],
                        lhsT=ones_e[:],
                        rhs=oh_t[:, j * P : (j + 1) * P],
                        start=(j == 0),
                        stop=(j == NBT - 1),
                    )
                nc.vector.tensor_copy(out=deg_sb[0:1, t * P : (t + 1) * P], in_=dps[:])
            # dis = sqrt(1/(deg+1)) broadcast to all partitions (in-place chain)
            nc.vector.tensor_scalar_add(deg_sb[:], deg_sb[:], 1.0)
            nc.vector.reciprocal(deg_sb[:], deg_sb[:])
            nc.scalar.activation(deg_sb[:], deg_sb[:], mybir.ActivationFunctionType.Sqrt)
            dis_bc = rp.tile([P, SHARD], f32, tag="dis_bc")
            for o in range(0, SHARD, 512):
                w = min(512, SHARD - o)
                bps = pp.tile([P, 512], f32, tag="mm")
                nc.tensor.matmul(out=bps[:, :w], lhsT=ones_k1[:],
                                 rhs=deg_sb[0:1, o : o + w], start=True, stop=True)
                nc.vector.tensor_copy(out=dis_bc[:, o : o + w], in_=bps[:, :w])

            # ---------- GCN layers
            hpools = [hA, hB]
            for layer in range(4):
                # phase A: hw' = (h @ W) * dis, transpose to node-major, stage AG in
                ag_in = dp.tile([SHARD, P], bf16, tag="ag_in")
                ag_out = dp.tile([NPAD, P], bf16, tag="ag_out")
                hw_tiles = []
                for t in range(TPC):
                    mm = pp.tile([P, P], f32, tag="mm")
                    nc.tensor.matmul(out=mm[:], lhsT=w_t[layer][:], rhs=h_cur[t][:],
                                     start=True, stop=True)
                    hw = hwp.tile([P, P], bf16, tag="hw")
                    nc.vector.tensor_tensor(
                        out=hw[:], in0=mm[:], in1=dis_bc[:, t * P : (t + 1) * P],
                        op=mybir.AluOpType.mult,
                    )
                    hw_tiles.append(hw)
                    tp = pst.tile([P, P], bf16, tag="mmt")
                    nc.tensor.transpose(out=tp[:], in_=hw[:], identity=id_t[:])
                    hwn = wk.tile([P, P], bf16, tag="hwn")
                    nc.vector.tensor_copy(out=hwn[:], in_=tp[:])
                    nc.sync.dma_start(out=ag_in[t * P : (t + 1) * P, :], in_=hwn[:])
                nc.gpsimd.collective_compute(
                    "AllGather",
                    mybir.AluOpType.bypass,
                    replica_groups=[list(range(NC))],
                    ins=[ag_in[:].opt()],
                    outs=[ag_out[:].opt()],
                )

                # phase C: gather + scatter-matmul + epilogue
                gat = {}

                def get_gather(stream, call):
                    key = (layer, stream, call)
                    if key in gat:
                        return gat[key]
                    g = gpool.tile([P, NIDX // P, F], bf16, tag="g")
                    if stream == 0:
                        base, win, it = 0, cfg.WLO, gil_t
                    else:
                        base, win, it = cfg.HI_BASE, cfg.WHI, gih_t
                    w = NIDX // 16
                    nc.gpsimd.dma_gather(
                        g[:],
                        ag_out[base : base + win, :],
                        it[:, call * w : (call + 1) * w],
                        NIDX, nidx_reg, F,
                        queue_num=(call % 4),
                    )
                    gat[key] = g
                    return g

                for t in range(TPC):
                    oh_t = ohp.tile([P, NBT * P], bf16, tag="oh")
                    nc.sync.dma_start(
                        out=oh_t[:].rearrange("p (b n) -> p b n", n=P),
                        in_=oh_v[:, t * NBT : (t + 1) * NBT, :],
                    )
                    agg = pp.tile([P, P], f32, tag="mm")
                    nb = 0
                    for cls in (0, 1):
                        BU = BLO if cls == 0 else BHI
                        for j in range(BU):
                            pos = (t * BU + j) * P
                            g = get_gather(cls, pos // NIDX)
                            sl = (pos % NIDX) // P
                            nc.tensor.matmul(
                                out=agg[:],
                                lhsT=g[:, sl, :],
                                rhs=oh_t[:, (cls * BLO + j) * P : (cls * BLO + j + 1) * P],
                                start=(nb == 0),
                                stop=(nb == NBT - 1),
                            )
                            nb += 1
                    # epilogue: h' = act(dis * (agg + hw') + b)
                    s1 = wk.tile([P, P], f32, tag="s1")
                    nc.vector.tensor_tensor(out=s1[:], in0=agg[:], in1=hw_tiles[t][:],
                                            op=mybir.AluOpType.add)
                    s2 = wk.tile([P, P], f32, tag="s2")
                    nc.vector.tensor_tensor(out=s2[:], in0=s1[:],
                                            in1=dis_bc[:, t * P : (t + 1) * P],
                                            op=mybir.AluOpType.mult)
                    hn = hpools[(layer + 1) % 2].tile([P, P], bf16, tag="h")
                    func = (mybir.ActivationFunctionType.Relu if layer < 3
                            else mybir.ActivationFunctionType.Identity)
                    nc.scalar.activation(hn[:], s2[:], func, bias=b_t[layer][:])
                    h_cur[t] = hn

            # ---------- final AG of h4 (node-major) for the head
            h4_in = dp.tile([SHARD, P], bf16, tag="ag_in")
            h4_tab = dp.tile([NPAD, P], bf16, tag="ag_out")
            for t in range(TPC):
                tp4 = pst.tile([P, P], bf16, tag="mmt")
                nc.tensor.transpose(out=tp4[:], in_=h_cur[t][:], identity=id_t[:])
                hn4 = wk.tile([P, P], bf16, tag="hwn")
                nc.vector.tensor_copy(out=hn4[:], in_=tp4[:])
                nc.sync.dma_start(out=h4_in[t * P : (t + 1) * P, :], in_=hn4[:])
            nc.gpsimd.collective_compute(
                "AllGather",
                mybir.AluOpType.bypass,
                replica_groups=[list(range(NC))],
                ins=[h4_in[:].opt()],
                outs=[h4_tab[:].opt()],
            )

            # ---------- head MLP over pair tiles
            hgat = {}

            def get_hgather(which, call):
                key = (which, call)
                if key in hgat:
                    return hgat[key]
                g = gpool.tile([P, 1, NIDXH], bf16, tag="hg")
                cls = pl.hcall_cls[call]
                hi_sel = (cls >> 1) if which == 0 else (cls & 1)
                base = cfg.HI_BASE if hi_sel else 0
                win = cfg.WHI if hi_sel else cfg.WLO
                it = hi0_t if which == 0 else hi1_t
                w = NIDXH // 16
                nc.gpsimd.dma_gather(
                    g[:],
                    h4_tab[base : base + win, :],
                    it[:, call * w : (call + 1) * w],
                    NIDXH, nidxh_reg, F,
                    transpose=True,
                    queue_num=(call % 4),
                )
                hgat[key] = g
                return g

            for pt in range(pl.PT):
                call = pt * P // NIDXH
                sl = (pt * P % NIDXH)
                g0 = get_hgather(0, call)
                g1 = get_hgather(1, call)
                z1p = pp.tile([P, P], f32, tag="mm")
                nc.tensor.matmul(out=z1p[:], lhsT=wl1_t[:, 0:P],
                                 rhs=g0[:, 0, sl : sl + P], start=True, stop=False)
                nc.tensor.matmul(out=z1p[:], lhsT=wl1_t[:, P : 2 * P],
                                 rhs=g1[:, 0, sl : sl + P], start=False, stop=True)
                z1 = wk.tile([P, P], bf16, tag="z1")
                nc.scalar.activation(z1[:], z1p[:], mybir.ActivationFunctionType.Relu,
                                     bias=bl1_t[:])
                z2p = ppr.tile([1, P], f32, tag="row")
                nc.tensor.matmul(out=z2p[:], lhsT=wl2_t[:], rhs=z1[:],
                                 start=True, stop=True)
                zrow = zp.tile([1, P], f32, tag="z")
                nc.vector.tensor_tensor(out=zrow[:], in0=z2p[:],
                                        in1=bl2_t[:].to_broadcast([1, P]),
                                        op=mybir.AluOpType.add)
                nc.sync.dma_start(out=zout[0:1, pt * P : (pt + 1) * P], in_=zrow[:])
    nc.compile()
    return nc


# ----------------------------------------------------------------------------
# Host wrapper


def _prep_inputs(cfg, pl, x, weights, core):
    (W0, b0, W1, b1, W2, b2, W3, b3, Wl1, bl1, Wl2, bl2) = weights
    SHARD = cfg.SHARD
    xp = np.zeros((cfg.NPAD, cfg.F), dtype=np.float32)
    xp[pl.ren[: x.shape[0]]] = x
    xT = xp[core * SHARD : (core + 1) * SHARD].T.astype(BF16)
    m = {
        "xT": np.ascontiguousarray(xT),
        "W0": W0.astype(BF16), "W1": W1.astype(BF16),
        "W2": W2.astype(BF16), "W3": W3.astype(BF16),
        "b0": b0.reshape(-1, 1).astype(np.float32),
        "b1": b1.reshape(-1, 1).astype(np.float32),
        "b2": b2.reshape(-1, 1).astype(np.float32),
        "b3": b3.reshape(-1, 1).astype(np.float32),
        "Wl1": Wl1.astype(BF16),
        "bl1": bl1.reshape(-1, 1).astype(np.float32),
        "Wl2": Wl2.reshape(-1, 1).astype(BF16),
        "bl2": bl2.reshape(1, 1).astype(np.float32),
        "onehot": pl.onehot[core],
        "gidx_lo": pl.gidx_lo[core],
        "gidx_hi": pl.gidx_hi[core],
        "id128": np.eye(cfg.F, dtype=np.float32).astype(BF16),
        "hidx0": pl.hidx0[core],
        "hidx1": pl.hidx1[core],
    }
    return m


def _unpack_head(cfg, pl, zouts):
    """Per-core zout [1, HTOT] -> global z [NP] in original pair order."""
    zs = []
    for c in range(cfg.NC):
        z = zouts[c].reshape(-1)
        parts = []
        off = 0
        for k in range(4):
            n = int(pl.hcounts[c, k])
            parts.append(z[off : off + n])
            off += pl.CLS_PAD[k]
        zc = np.concatenate(parts)
        orig = np.empty(pl.PPC, dtype=np.float32)
        orig[pl.hperm[c]] = zc
        zs.append(orig)
    return np.concatenate(zs)


def run(cfg, x, edge_index, pairs, weights, trace=False):
    pl = make_plan(cfg, edge_index, pairs)
    nc = build_bass(cfg, pl)
    in_maps = [_prep_inputs(cfg, pl, x, weights, c) for c in range(cfg.NC)]
    res = run_bass_kernel_spmd(nc, in_maps, core_ids=list(range(cfg.NC)), trace=trace)
    z = _unpack_head(cfg, pl, [res.results[c]["zout"] for c in range(cfg.NC)])
    return z, res


def kernel(x, edge_index, pos_edges_train, neg_edges_train, pos_edges_test,
           neg_edges_test, W0, b0, W1, b1, W2, b2, W3, b3, Wl1, bl1, Wl2, bl2):
    cfg = full_cfg()
    pairs = np.concatenate([
        np.asarray(pos_edges_train).T, np.asarray(neg_edges_train).T,
        np.asarray(pos_edges_test).T, np.asarray(neg_edges_test).T,
    ], axis=0)
    weights = (np.asarray(W0), np.asarray(b0), np.asarray(W1), np.asarray(b1),
               np.asarray(W2), np.asarray(b2), np.asarray(W3), np.asarray(b3),
               np.asarray(Wl1), np.asarray(bl1), np.asarray(Wl2), np.asarray(bl2))
    trace = bool(int(os.environ.get("GCN_TRACE", "0")))
    z, res = run(cfg, np.asarray(x), np.asarray(edge_index), pairs, weights,
                 trace=trace)
    global LAST_EXEC_NS
    LAST_EXEC_NS = res.exec_time_ns
    n_train = pos_edges_train.shape[1] + neg_edges_train.shape[1]
    return z[:n_train].astype(np.float32), z[n_train:].astype(np.float32)
